# revision 6
# baseline (speedup 1.0000x reference)
"""CoherenceGuidedAttention Trainium2 Bass kernel.

Sharding: data-parallel over batch B=8 -> one batch per NeuronCore (8 cores).
All parameters replicated; no collectives.

Per-core algorithm (flash-style, channel-major "layout B"):
  X_vv, X_vh       : [C=128, N=1024] SBUF (natural layout of [C,H,W] input)
  Q^T = temp*(Wq X_vv + bq), K^T = Wk X_vh + bk   (pair-padded head layout)
  V'  = (X_vh^T Wv + bv | ones)  token-major with per-head 17-col groups
  S^T(h) = K_h Q_h^T   computed per (head, m-tile) into PSUM [128,1024]
           + rank-1 accumulate  c*w w^T  (c=(1-g)/g) via K=1 matmuls
  expS = exp(g * S^T_blend)  on ACT directly from PSUM (scale = per-partition g)
  PV   = V'_h^T expS  accumulated over m-chunks -> [17, 512] (row 16 = denom)
  Aout^T[h] = PV[0:16] * (1/denom)  (fast reciprocal + partition broadcast)
  Y^T = Wo Aout^T + bo + X_vv  -> output [C, N]

Head layout for PE row-tiling: Q^T/K^T stored as 4 pair-tiles [128, 1024];
pair t holds head 2t at partitions 0:16 and head 2t+1 at partitions 64:80,
so the two heads' QK^T matmuls run concurrently in PE row-strips 0 and 64.
"""

import sys

if "/opt/trn_rl_repo" not in sys.path:
    sys.path.insert(0, "/opt/trn_rl_repo")

import numpy as np

B = 8
C = 128
HW_N = 1024  # H*W tokens
NH = 8       # heads
HD = 16      # head dim
NCORES = 8
NPAIR = 4    # head pairs
MT = 8       # m tiles of 128 tokens
NCHUNK = 512  # fp32 moving-operand max

_CACHE: dict = {}


def build_program():
    """Build (once) the SPMD Bass program for one core."""
    if "nc" in _CACHE:
        return _CACHE["nc"]

    from concourse import bacc
    import concourse.mybir as mybir
    import concourse.tile as tile

    f32 = mybir.dt.float32
    AF = mybir.ActivationFunctionType
    OP = mybir.AluOpType

    nc = bacc.Bacc("TRN2", target_bir_lowering=False, debug=False)

    # ---- DRAM I/O ----------------------------------------------------------
    x_vv_d = nc.dram_tensor("x_vv", [C, HW_N], f32, kind="ExternalInput")
    x_vh_d = nc.dram_tensor("x_vh", [C, HW_N], f32, kind="ExternalInput")
    coh_d = nc.dram_tensor("coh", [1, HW_N], f32, kind="ExternalInput")
    wq_d = nc.dram_tensor("wq", [C, NPAIR * C], f32, kind="ExternalInput")
    wk_d = nc.dram_tensor("wk", [C, NPAIR * C], f32, kind="ExternalInput")
    bq_d = nc.dram_tensor("bq", [C, NPAIR], f32, kind="ExternalInput")
    bk_d = nc.dram_tensor("bk", [C, NPAIR], f32, kind="ExternalInput")
    wv_d = nc.dram_tensor("wv", [C, C], f32, kind="ExternalInput")
    bv_d = nc.dram_tensor("bv", [C, C], f32, kind="ExternalInput")
    woa_d = nc.dram_tensor("woa", [C, C], f32, kind="ExternalInput")
    wob_d = nc.dram_tensor("wob", [C, C], f32, kind="ExternalInput")
    bo_d = nc.dram_tensor("bo", [C, 1], f32, kind="ExternalInput")
    g1vv_d = nc.dram_tensor("g1vv", [C, 64], f32, kind="ExternalInput")
    g1vh_d = nc.dram_tensor("g1vh", [C, 64], f32, kind="ExternalInput")
    g1st_d = nc.dram_tensor("g1st", [3, 64], f32, kind="ExternalInput")
    g1b_d = nc.dram_tensor("g1b", [1, 64], f32, kind="ExternalInput")
    g2w_d = nc.dram_tensor("g2w", [64, 32], f32, kind="ExternalInput")
    g2b_d = nc.dram_tensor("g2b", [1, 32], f32, kind="ExternalInput")
    g3w_d = nc.dram_tensor("g3w", [32, 1], f32, kind="ExternalInput")
    g3b_d = nc.dram_tensor("g3b", [1, 1], f32, kind="ExternalInput")
    y_d = nc.dram_tensor("y", [C, HW_N], f32, kind="ExternalOutput")

    with tile.TileContext(nc) as tc:
        with (
            tc.tile_pool(name="persist", bufs=1) as persist,
            tc.tile_pool(name="qkbuf", bufs=1) as qkbuf,
            tc.tile_pool(name="expp", bufs=6) as expp,
            tc.tile_pool(name="small", bufs=8) as small,
            tc.tile_pool(name="rdp", bufs=4) as rdp,
            tc.tile_pool(name="rdbp", bufs=4) as rdbp,
            tc.tile_pool(name="ps_s", bufs=2, space="PSUM") as ps_s,
            tc.tile_pool(name="ps_pv", bufs=4, space="PSUM") as ps_pv,
        ):
            # ---- load inputs ------------------------------------------------
            xvv = persist.tile([C, HW_N], f32)
            xvh = persist.tile([C, HW_N], f32)
            coh = persist.tile([1, HW_N], f32)
            wq = persist.tile([C, NPAIR, C], f32)
            wk = persist.tile([C, NPAIR, C], f32)
            bq = persist.tile([C, NPAIR], f32)
            bk = persist.tile([C, NPAIR], f32)
            wv = persist.tile([C, C], f32)
            bvb = persist.tile([C, C], f32)
            woa = persist.tile([C, C], f32)
            wob = persist.tile([C, C], f32)
            bo = persist.tile([C, 1], f32)
            g1vv = persist.tile([C, 64], f32)
            g1vh = persist.tile([C, 64], f32)
            g1st = persist.tile([3, 64], f32)
            g1b = persist.tile([1, 64], f32)
            g2w = persist.tile([64, 32], f32)
            g2b = persist.tile([1, 32], f32)
            g3w = persist.tile([32, 1], f32)
            g3b = persist.tile([1, 1], f32)

            nc.sync.dma_start(out=xvv, in_=x_vv_d[:, :])
            nc.sync.dma_start(out=xvh, in_=x_vh_d[:, :])
            nc.sync.dma_start(out=coh, in_=coh_d[:, :])
            nc.sync.dma_start(out=wq, in_=wq_d.ap().rearrange("p (t m) -> p t m", t=NPAIR))
            nc.sync.dma_start(out=wk, in_=wk_d.ap().rearrange("p (t m) -> p t m", t=NPAIR))
            nc.sync.dma_start(out=bq, in_=bq_d[:, :])
            nc.sync.dma_start(out=bk, in_=bk_d[:, :])
            nc.sync.dma_start(out=wv, in_=wv_d[:, :])
            nc.sync.dma_start(out=bvb, in_=bv_d[:, :])
            nc.sync.dma_start(out=woa, in_=woa_d[:, :])
            nc.sync.dma_start(out=wob, in_=wob_d[:, :])
            nc.sync.dma_start(out=bo, in_=bo_d[:, :])
            nc.sync.dma_start(out=g1vv, in_=g1vv_d[:, :])
            nc.sync.dma_start(out=g1vh, in_=g1vh_d[:, :])
            nc.sync.dma_start(out=g1st, in_=g1st_d[:, :])
            nc.sync.dma_start(out=g1b, in_=g1b_d[:, :])
            nc.sync.dma_start(out=g2w, in_=g2w_d[:, :])
            nc.sync.dma_start(out=g2b, in_=g2b_d[:, :])
            nc.sync.dma_start(out=g3w, in_=g3w_d[:, :])
            nc.sync.dma_start(out=g3b, in_=g3b_d[:, :])

            ones = persist.tile([1, C], f32)
            nc.vector.memset(ones, 1.0)
            one1 = ones[:, 0:1]

            # ---- coherence weights w + stats -------------------------------
            cmx = small.tile([1, 1], f32)
            nc.vector.tensor_reduce(out=cmx, in_=coh, axis=mybir.AxisListType.X, op=OP.max)
            cmn = small.tile([1, 1], f32)
            nc.vector.tensor_reduce(out=cmn, in_=coh, axis=mybir.AxisListType.X, op=OP.min)
            rng = small.tile([1, 1], f32)
            # (cmax + 1e-8) - cmin
            nc.vector.scalar_tensor_tensor(
                out=rng, in0=cmx, scalar=1e-8, in1=cmn, op0=OP.add, op1=OP.subtract
            )
            rcp = small.tile([1, 1], f32)
            nc.vector.reciprocal(out=rcp, in_=rng)
            w_sb = persist.tile([1, HW_N], f32)
            nc.vector.tensor_scalar(
                out=w_sb, in0=coh, scalar1=cmn, scalar2=rcp, op0=OP.subtract, op1=OP.mult
            )

            # stats: s1 = sum(w), s2 = sum(w^2), wmx = max(w)
            s1 = small.tile([1, 1], f32)
            nc.vector.tensor_reduce(out=s1, in_=w_sb, axis=mybir.AxisListType.X, op=OP.add)
            w2 = small.tile([1, HW_N], f32, tag="wrow")
            nc.vector.tensor_mul(out=w2, in0=w_sb, in1=w_sb)
            s2 = small.tile([1, 1], f32)
            nc.vector.tensor_reduce(out=s2, in_=w2, axis=mybir.AxisListType.X, op=OP.add)
            wmx = small.tile([1, 1], f32)
            nc.vector.tensor_reduce(out=wmx, in_=w_sb, axis=mybir.AxisListType.X, op=OP.max)
            # std = sqrt(s2/N - (s1/N)^2) via exp(0.5*ln(var))
            m1 = small.tile([1, 1], f32)
            nc.vector.tensor_scalar_mul(out=m1, in0=s1, scalar1=1.0 / HW_N)
            msq = small.tile([1, 1], f32)
            nc.vector.tensor_mul(out=msq, in0=m1, in1=m1)
            var = small.tile([1, 1], f32)
            nc.vector.scalar_tensor_tensor(
                out=var, in0=s2, scalar=1.0 / HW_N, in1=msq, op0=OP.mult, op1=OP.subtract
            )
            lnv = small.tile([1, 1], f32)
            nc.scalar.activation(out=lnv, in_=var, func=AF.Ln)
            std = small.tile([1, 1], f32)
            nc.scalar.activation(out=std, in_=lnv, func=AF.Exp, scale=0.5)
            # stats row [1,3] = [s1, std, max]  (g1st row 0 is pre-scaled by 1/N)
            strow = small.tile([1, 3], f32)
            nc.vector.tensor_copy(out=strow[:, 0:1], in_=s1)
            nc.vector.tensor_copy(out=strow[:, 1:2], in_=std)
            nc.vector.tensor_copy(out=strow[:, 2:3], in_=wmx)
            ps_st = ps_pv.tile([C, NCHUNK], f32, tag="pv")
            nc.tensor.matmul(out=ps_st[0:3, 0:1], lhsT=strow, rhs=one1,
                             start=True, stop=True, tile_position=(0, 0))
            stcol = small.tile([3, 1], f32)
            nc.vector.tensor_copy(out=stcol, in_=ps_st[0:3, 0:1])

            # ---- gate MLP ---------------------------------------------------
            vvs = small.tile([C, 1], f32, tag="col")
            nc.vector.tensor_reduce(out=vvs, in_=xvv, axis=mybir.AxisListType.X, op=OP.add)
            vhs = small.tile([C, 1], f32, tag="col")
            nc.vector.tensor_reduce(out=vhs, in_=xvh, axis=mybir.AxisListType.X, op=OP.add)

            ps_g = ps_pv.tile([C, NCHUNK], f32, tag="pv")
            h1p = ps_g[0:64, 0:1]
            nc.tensor.matmul(out=h1p, lhsT=g1vv, rhs=vvs, start=True, stop=False,
                             tile_position=(0, 0))
            nc.tensor.matmul(out=h1p, lhsT=g1vh, rhs=vhs, start=False, stop=False,
                             tile_position=(0, 0))
            nc.tensor.matmul(out=h1p, lhsT=g1st, rhs=stcol, start=False, stop=False,
                             tile_position=(0, 0))
            nc.tensor.matmul(out=h1p, lhsT=g1b, rhs=one1, start=False, stop=True,
                             tile_position=(0, 0))
            h1 = small.tile([64, 1], f32, tag="col")
            nc.scalar.activation(out=h1, in_=h1p, func=AF.Relu)

            ps_g2 = ps_pv.tile([C, NCHUNK], f32, tag="pv")
            h2p = ps_g2[0:32, 0:1]
            nc.tensor.matmul(out=h2p, lhsT=g2w, rhs=h1, start=True, stop=False,
                             tile_position=(0, 0))
            nc.tensor.matmul(out=h2p, lhsT=g2b, rhs=one1, start=False, stop=True,
                             tile_position=(0, 0))
            h2 = small.tile([32, 1], f32, tag="col")
            nc.scalar.activation(out=h2, in_=h2p, func=AF.Relu)

            ps_g3 = ps_pv.tile([C, NCHUNK], f32, tag="pv")
            zp = ps_g3[0:1, 0:1]
            nc.tensor.matmul(out=zp, lhsT=g3w, rhs=h2, start=True, stop=False,
                             tile_position=(0, 0))
            nc.tensor.matmul(out=zp, lhsT=g3b, rhs=one1, start=False, stop=True,
                             tile_position=(0, 0))
            # g = sigmoid(z) = 1/(1+exp(-z))
            ez = small.tile([1, 1], f32)
            nc.scalar.activation(out=ez, in_=zp, func=AF.Exp, scale=-1.0)
            gden = small.tile([1, 1], f32)
            nc.vector.tensor_scalar_add(out=gden, in0=ez, scalar1=1.0)
            gsc = small.tile([1, 1], f32)
            nc.vector.reciprocal(out=gsc, in_=gden)
            # broadcast g to all partitions
            ps_gb = ps_pv.tile([C, NCHUNK], f32, tag="pv")
            nc.tensor.matmul(out=ps_gb[:, 0:1], lhsT=ones, rhs=gsc,
                             start=True, stop=True, tile_position=(0, 0))
            g_col = persist.tile([C, 1], f32)
            nc.vector.tensor_copy(out=g_col, in_=ps_gb[:, 0:1])
            # c = (1-g)/g = 1/g - 1  (per partition)
            rg = small.tile([C, 1], f32, tag="col")
            nc.vector.reciprocal(out=rg, in_=g_col)
            c_col = persist.tile([C, 1], f32)
            nc.vector.tensor_scalar_add(out=c_col, in0=rg, scalar1=-1.0)

            # w replicated on all partitions; wc = c * w
            ps_wr = ps_s.tile([C, HW_N], f32, tag="s")
            for ncb in range(2):
                sl = slice(ncb * NCHUNK, (ncb + 1) * NCHUNK)
                nc.tensor.matmul(out=ps_wr[:, sl], lhsT=ones, rhs=w_sb[:, sl],
                                 start=True, stop=True, tile_position=(0, 0))
            w_rep = persist.tile([C, HW_N], f32)
            nc.vector.tensor_copy(out=w_rep, in_=ps_wr)
            wc_rep = persist.tile([C, HW_N], f32)
            nc.vector.tensor_scalar_mul(out=wc_rep, in0=w_rep, scalar1=c_col)

            # ---- V' projection (token-major, 17-col head groups + ones) ----
            # vp[p, mc, h, 0] = 1;  vp[p, mc, h, 1:17] = V_seq[mc*128+p, 16h:16h+16]
            # (ones first so the PV denominator row lands on the 32-aligned
            #  strip base - engine APs must start at partition 0/32/64/96)
            vp = persist.tile([C, MT, NH, HD + 1], f32)
            nc.vector.memset(vp[:, :, :, 0:1], 1.0)
            for gp in range(2):
                ps_v = ps_pv.tile([C, NCHUNK], f32, tag="pv")
                for i in range(4):
                    mc = 4 * gp + i
                    nc.tensor.matmul(
                        out=ps_v[:, i * C : (i + 1) * C],
                        lhsT=xvh[:, mc * C : (mc + 1) * C],
                        rhs=wv,
                        start=True, stop=True, tile_position=(0, 0),
                    )
                for i in range(4):
                    mc = 4 * gp + i
                    nc.vector.tensor_add(
                        out=vp[:, mc, :, 1 : HD + 1],
                        in0=ps_v[:, i * C : (i + 1) * C].rearrange(
                            "p (h d) -> p h d", h=NH
                        ),
                        in1=bvb.rearrange("p (h d) -> p h d", h=NH),
                    )

            # ---- Q^T / K^T projections (pair-padded head layout) -----------
            qt = qkbuf.tile([C, NPAIR, HW_N], f32)
            kt = qkbuf.tile([C, NPAIR, HW_N], f32)
            for t in range(NPAIR):
                ps_q = ps_s.tile([C, HW_N], f32, tag="s")
                for ncb in range(2):
                    sl = slice(ncb * NCHUNK, (ncb + 1) * NCHUNK)
                    nc.tensor.matmul(out=ps_q[:, sl], lhsT=wq[:, t, :], rhs=xvv[:, sl],
                                     start=True, stop=True, tile_position=(0, 0))
                nc.vector.tensor_scalar_add(out=qt[:, t, :], in0=ps_q, scalar1=bq[:, t : t + 1])
            for t in range(NPAIR):
                ps_k = ps_s.tile([C, HW_N], f32, tag="s")
                for ncb in range(2):
                    sl = slice(ncb * NCHUNK, (ncb + 1) * NCHUNK)
                    nc.tensor.matmul(out=ps_k[:, sl], lhsT=wk[:, t, :], rhs=xvh[:, sl],
                                     start=True, stop=True, tile_position=(0, 0))
                nc.vector.tensor_scalar_add(out=kt[:, t, :], in0=ps_k, scalar1=bk[:, t : t + 1])

            # ---- attention: S^T -> exp -> PV, flash-style ------------------
            aoutA = persist.tile([C, HW_N], f32)
            aoutB = persist.tile([C, HW_N], f32)
            nc.vector.memset(aoutA, 0.0)
            nc.vector.memset(aoutB, 0.0)
            pv_tiles = {}
            for t in range(NPAIR):
                if t % 2 == 0:
                    for ncb in range(2):
                        pv_tiles[ncb] = ps_pv.tile(
                            [C, NCHUNK], f32, tag="pv", name=f"pv_{t}_{ncb}"
                        )
                for mt_i in range(MT):
                    msl = slice(mt_i * C, (mt_i + 1) * C)
                    es_tiles = []
                    for si, s in enumerate((0, 64)):
                        h = 2 * t + si
                        ps = ps_s.tile([C, HW_N], f32, tag="s")
                        for ncb in range(2):
                            sl = slice(ncb * NCHUNK, (ncb + 1) * NCHUNK)
                            nc.tensor.matmul(
                                out=ps[:, sl],
                                lhsT=kt[s : s + HD, t, msl],
                                rhs=qt[s : s + HD, t, sl],
                                start=True, stop=False, tile_position=(s, 0),
                            )
                            nc.tensor.matmul(
                                out=ps[:, sl],
                                lhsT=wc_rep[s : s + 1, msl],
                                rhs=w_rep[s : s + 1, sl],
                                start=False, stop=True, tile_position=(s, 0),
                            )
                        es = expp.tile([C, HW_N], f32, tag="es")
                        nc.scalar.activation(out=es, in_=ps, func=AF.Exp, scale=g_col)
                        es_tiles.append((h, es))
                    for h, es in es_tiles:
                        j = h % 4
                        for ncb in range(2):
                            sl = slice(ncb * NCHUNK, (ncb + 1) * NCHUNK)
                            # skip_group_check: the sim's physical group-check
                            # array mis-addresses sub-bank partition offsets;
                            # disjoint-partition groups are safe on HW
                            # (per-element has_written, per-partition zeroing).
                            nc.tensor.matmul(
                                out=pv_tiles[ncb][32 * j : 32 * j + HD + 1, :],
                                lhsT=vp[:, mt_i, h, :],
                                rhs=es[:, sl],
                                start=(mt_i == 0), stop=(mt_i == MT - 1),
                                tile_position=(0, 32 * j),
                                skip_group_check=True,
                            )
                if t % 2 == 1:
                    # normalize the 4 finished heads
                    for si in range(4):
                        h = 4 * (t // 2) + si
                        j = h % 4
                        for ncb in range(2):
                            pvt = pv_tiles[ncb]
                            rd = rdp.tile([1, NCHUNK], f32, tag="rd")
                            nc.vector.reciprocal_approx_fast(
                                out=rd, in_=pvt[32 * j : 32 * j + 1, :]
                            )
                            rdb = rdbp.tile([HD + 1, NCHUNK], f32, tag="rdb")
                            nc.gpsimd.partition_broadcast(rdb, rd)
                            # row 32j is denom*recip(denom) ~= 1 junk; it hits a
                            # zero row of the padded Wo so it never contributes.
                            dst = aoutA if h < 4 else aoutB
                            nc.vector.tensor_mul(
                                out=dst[32 * j : 32 * j + HD + 1,
                                        ncb * NCHUNK : (ncb + 1) * NCHUNK],
                                in0=pvt[32 * j : 32 * j + HD + 1, :],
                                in1=rdb,
                            )

            # ---- output projection + bias + residual -----------------------
            ps_y = ps_s.tile([C, HW_N], f32, tag="s")
            for ncb in range(2):
                sl = slice(ncb * NCHUNK, (ncb + 1) * NCHUNK)
                nc.tensor.matmul(out=ps_y[:, sl], lhsT=woa, rhs=aoutA[:, sl],
                                 start=True, stop=False, tile_position=(0, 0))
                nc.tensor.matmul(out=ps_y[:, sl], lhsT=wob, rhs=aoutB[:, sl],
                                 start=False, stop=True, tile_position=(0, 0))
            y_sb = persist.tile([C, HW_N], f32)
            # (ps_y + bo) + x_vv
            nc.vector.scalar_tensor_tensor(
                out=y_sb, in0=ps_y, scalar=bo, in1=xvv, op0=OP.add, op1=OP.add
            )
            nc.sync.dma_start(out=y_d[:, :], in_=y_sb)

    nc.compile()
    _CACHE["nc"] = nc
    return nc


def make_in_maps(inputs: dict) -> list[dict]:
    """Host-side prep: shard over batch, pre-transpose/pad the small weights."""
    f32 = np.float32
    vv = np.ascontiguousarray(inputs["vv_features"], dtype=f32)
    vh = np.ascontiguousarray(inputs["vh_features"], dtype=f32)
    coh = np.ascontiguousarray(inputs["coherence_matrix"], dtype=f32)
    Wq = np.asarray(inputs["Wq"], f32)
    bq = np.asarray(inputs["bq"], f32)
    Wk = np.asarray(inputs["Wk"], f32)
    bk = np.asarray(inputs["bk"], f32)
    Wv = np.asarray(inputs["Wv"], f32)
    bv = np.asarray(inputs["bv"], f32)
    Wo = np.asarray(inputs["Wo"], f32)
    bo = np.asarray(inputs["bo"], f32)
    temp = float(np.asarray(inputs["temperature"], f32).reshape(-1)[0])
    g1w = np.asarray(inputs["g1w"], f32)
    g1b = np.asarray(inputs["g1b"], f32)
    g2w = np.asarray(inputs["g2w"], f32)
    g2b = np.asarray(inputs["g2b"], f32)
    g3w = np.asarray(inputs["g3w"], f32)
    g3b = np.asarray(inputs["g3b"], f32)

    def _wo_pad(Wo_, grp):
        # lhsT [c_in_padded=128, c_out=128]: strip j row d holds Wo column for
        # channel 16*(4*grp+j)+d; pad rows (d>=16) are zero.
        wp = np.zeros((C, C), f32)
        for j in range(4):
            ch0 = HD * (4 * grp + j)
            wp[32 * j + 1 : 32 * j + 1 + HD, :] = Wo_[:, ch0 : ch0 + HD].T
        return np.ascontiguousarray(wp)

    def pad_pair(Wt, bt):
        # -> lhsT tiles [C, NPAIR, C] flattened to [C, NPAIR*C]; bias [C, NPAIR]
        wpad = np.zeros((NPAIR, C, C), f32)   # [t, c_in, m]
        bpad = np.zeros((C, NPAIR), f32)
        for t in range(NPAIR):
            wpad[t, :, 0:HD] = Wt[2 * t * HD : (2 * t + 1) * HD, :].T
            wpad[t, :, 64 : 64 + HD] = Wt[(2 * t + 1) * HD : (2 * t + 2) * HD, :].T
            bpad[0:HD, t] = bt[2 * t * HD : (2 * t + 1) * HD]
            bpad[64 : 64 + HD, t] = bt[(2 * t + 1) * HD : (2 * t + 2) * HD]
        wflat = np.ascontiguousarray(wpad.transpose(1, 0, 2).reshape(C, NPAIR * C))
        return wflat, np.ascontiguousarray(bpad)

    wq_h, bq_h = pad_pair(Wq * temp, bq * temp)
    wk_h, bk_h = pad_pair(Wk, bk)

    shared = {
        "wq": wq_h, "bq": bq_h, "wk": wk_h, "bk": bk_h,
        "wv": np.ascontiguousarray(Wv.T), "bv": np.ascontiguousarray(np.tile(bv, (C, 1))),
        "woa": _wo_pad(Wo, 0), "wob": _wo_pad(Wo, 1),
        "bo": np.ascontiguousarray(bo[:, None]),
        "g1vv": np.ascontiguousarray(g1w[:, :C].T / HW_N),
        "g1vh": np.ascontiguousarray(g1w[:, C : 2 * C].T / HW_N),
        "g1st": np.ascontiguousarray(g1w[:, 2 * C : 2 * C + 3].T * np.array([1.0 / HW_N, 1.0, 1.0], f32)[:, None]),
        "g1b": np.ascontiguousarray(g1b[None, :]),
        "g2w": np.ascontiguousarray(g2w.T), "g2b": np.ascontiguousarray(g2b[None, :]),
        "g3w": np.ascontiguousarray(g3w.T), "g3b": np.ascontiguousarray(g3b[None, :]),
    }
    in_maps = []
    for b in range(B):
        m = dict(shared)
        m["x_vv"] = np.ascontiguousarray(vv[b].reshape(C, HW_N))
        m["x_vh"] = np.ascontiguousarray(vh[b].reshape(C, HW_N))
        m["coh"] = np.ascontiguousarray(coh[b].reshape(1, HW_N))
        in_maps.append(m)
    return in_maps


def kernel(**inputs) -> np.ndarray:
    nc = build_program()
    in_maps = make_in_maps(inputs)
    from concourse.bass_utils import run_bass_kernel_spmd

    res = run_bass_kernel_spmd(nc, in_maps, core_ids=list(range(NCORES)))
    out = np.stack([res.results[i]["y"].reshape(C, 32, 32) for i in range(B)])
    return np.ascontiguousarray(out.astype(np.float32))


# revision 10
# speedup vs baseline: 2.7662x; 2.7662x over previous
"""CoherenceGuidedAttention Trainium2 Bass kernel.

Sharding: data-parallel over batch B=8 -> one batch per NeuronCore (8 cores).
All parameters replicated; no collectives.

Per-core algorithm (flash-style, channel-major "layout B"):
  X_vv, X_vh       : [C=128, N=1024] SBUF (natural layout of [C,H,W] input)
  Q^T = temp*(Wq X_vv + bq), K^T = Wk X_vh + bk   (pair-padded head layout)
  V'  = (X_vh^T Wv + bv | ones)  token-major with per-head 17-col groups
  S^T(h) = K_h Q_h^T   computed per (head, m-tile) into PSUM [128,1024]
           + rank-1 accumulate  c*w w^T  (c=(1-g)/g) via K=1 matmuls
  expS = exp(g * S^T_blend)  on ACT directly from PSUM (scale = per-partition g)
  PV   = V'_h^T expS  accumulated over m-chunks -> [17, 512] (row 16 = denom)
  Aout^T[h] = PV[0:16] * (1/denom)  (fast reciprocal + partition broadcast)
  Y^T = Wo Aout^T + bo + X_vv  -> output [C, N]

Head layout for PE row-tiling: Q^T/K^T stored as 4 pair-tiles [128, 1024];
pair t holds head 2t at partitions 0:16 and head 2t+1 at partitions 64:80,
so the two heads' QK^T matmuls run concurrently in PE row-strips 0 and 64.
"""

import sys

if "/opt/trn_rl_repo" not in sys.path:
    sys.path.insert(0, "/opt/trn_rl_repo")

import numpy as np

B = 8
C = 128
HW_N = 1024  # H*W tokens
NH = 8       # heads
HD = 16      # head dim
NCORES = 8
NPAIR = 4    # head pairs
MT = 8       # m tiles of 128 tokens
NCHUNK = 512  # fp32 moving-operand max

_CACHE: dict = {}


def build_program():
    """Build (once) the SPMD Bass program for one core."""
    if "nc" in _CACHE:
        return _CACHE["nc"]

    from concourse import bacc
    import concourse.mybir as mybir
    import concourse.tile as tile

    f32 = mybir.dt.float32
    f32r = mybir.dt.float32r
    AF = mybir.ActivationFunctionType
    OP = mybir.AluOpType

    nc = bacc.Bacc("TRN2", target_bir_lowering=False, debug=False)

    # ---- DRAM I/O ----------------------------------------------------------
    x_vv_d = nc.dram_tensor("x_vv", [C, HW_N], f32, kind="ExternalInput")
    x_vh_d = nc.dram_tensor("x_vh", [C, HW_N], f32, kind="ExternalInput")
    coh_d = nc.dram_tensor("coh", [1, HW_N], f32, kind="ExternalInput")
    wq_d = nc.dram_tensor("wq", [C, NPAIR * C], f32, kind="ExternalInput")
    wk_d = nc.dram_tensor("wk", [C, NPAIR * C], f32, kind="ExternalInput")
    bq_d = nc.dram_tensor("bq", [C, NPAIR], f32, kind="ExternalInput")
    bk_d = nc.dram_tensor("bk", [C, NPAIR], f32, kind="ExternalInput")
    wv_d = nc.dram_tensor("wv", [C, C], f32, kind="ExternalInput")
    bv_d = nc.dram_tensor("bv", [C, C], f32, kind="ExternalInput")
    woa_d = nc.dram_tensor("woa", [C, C], f32, kind="ExternalInput")
    wob_d = nc.dram_tensor("wob", [C, C], f32, kind="ExternalInput")
    bo_d = nc.dram_tensor("bo", [C, 1], f32, kind="ExternalInput")
    g1vv_d = nc.dram_tensor("g1vv", [C, 64], f32, kind="ExternalInput")
    g1vh_d = nc.dram_tensor("g1vh", [C, 64], f32, kind="ExternalInput")
    g1st_d = nc.dram_tensor("g1st", [3, 64], f32, kind="ExternalInput")
    g1b_d = nc.dram_tensor("g1b", [1, 64], f32, kind="ExternalInput")
    g2w_d = nc.dram_tensor("g2w", [64, 32], f32, kind="ExternalInput")
    g2b_d = nc.dram_tensor("g2b", [1, 32], f32, kind="ExternalInput")
    g3w_d = nc.dram_tensor("g3w", [32, 1], f32, kind="ExternalInput")
    g3b_d = nc.dram_tensor("g3b", [1, 1], f32, kind="ExternalInput")
    y_d = nc.dram_tensor("y", [C, HW_N], f32, kind="ExternalOutput")

    with tile.TileContext(nc) as tc:
        with (
            tc.tile_pool(name="persist", bufs=1) as persist,
            tc.tile_pool(name="qkbuf", bufs=1) as qkbuf,
            tc.tile_pool(name="expp", bufs=6) as expp,
            tc.tile_pool(name="small", bufs=8) as small,
            tc.tile_pool(name="rdp", bufs=4) as rdp,
            tc.tile_pool(name="rdbp", bufs=4) as rdbp,
            tc.tile_pool(name="ps_s", bufs=2, space="PSUM") as ps_s,
            tc.tile_pool(name="ps_pv", bufs=4, space="PSUM") as ps_pv,
        ):
            # ---- load inputs ------------------------------------------------
            xvv = persist.tile([C, HW_N], f32)
            xvh = persist.tile([C, HW_N], f32)
            coh = persist.tile([1, HW_N], f32)
            wq = persist.tile([C, NPAIR, C], f32)
            wk = persist.tile([C, NPAIR, C], f32)
            bq = persist.tile([C, NPAIR], f32)
            bk = persist.tile([C, NPAIR], f32)
            wv = persist.tile([C, C], f32)
            bvb = persist.tile([C, C], f32)
            woa = persist.tile([C, C], f32)
            wob = persist.tile([C, C], f32)
            bo = persist.tile([C, 1], f32)
            g1vv = persist.tile([C, 64], f32)
            g1vh = persist.tile([C, 64], f32)
            g1st = persist.tile([3, 64], f32)
            g1b = persist.tile([1, 64], f32)
            g2w = persist.tile([64, 32], f32)
            g2b = persist.tile([1, 32], f32)
            g3w = persist.tile([32, 1], f32)
            g3b = persist.tile([1, 1], f32)

            nc.sync.dma_start(out=xvv, in_=x_vv_d[:, :])
            nc.sync.dma_start(out=xvh, in_=x_vh_d[:, :])
            nc.sync.dma_start(out=coh, in_=coh_d[:, :])
            nc.sync.dma_start(out=wq, in_=wq_d.ap().rearrange("p (t m) -> p t m", t=NPAIR))
            nc.sync.dma_start(out=wk, in_=wk_d.ap().rearrange("p (t m) -> p t m", t=NPAIR))
            nc.sync.dma_start(out=bq, in_=bq_d[:, :])
            nc.sync.dma_start(out=bk, in_=bk_d[:, :])
            nc.sync.dma_start(out=wv, in_=wv_d[:, :])
            nc.sync.dma_start(out=bvb, in_=bv_d[:, :])
            nc.sync.dma_start(out=woa, in_=woa_d[:, :])
            nc.sync.dma_start(out=wob, in_=wob_d[:, :])
            nc.sync.dma_start(out=bo, in_=bo_d[:, :])
            nc.sync.dma_start(out=g1vv, in_=g1vv_d[:, :])
            nc.sync.dma_start(out=g1vh, in_=g1vh_d[:, :])
            nc.sync.dma_start(out=g1st, in_=g1st_d[:, :])
            nc.sync.dma_start(out=g1b, in_=g1b_d[:, :])
            nc.sync.dma_start(out=g2w, in_=g2w_d[:, :])
            nc.sync.dma_start(out=g2b, in_=g2b_d[:, :])
            nc.sync.dma_start(out=g3w, in_=g3w_d[:, :])
            nc.sync.dma_start(out=g3b, in_=g3b_d[:, :])

            ones = persist.tile([1, C], f32)
            nc.vector.memset(ones, 1.0)
            one1 = ones[:, 0:1]
            ones_col = persist.tile([C, 1], f32)
            nc.vector.memset(ones_col, 1.0)
            zero_col = persist.tile([C, 1], f32)
            nc.vector.memset(zero_col, 0.0)

            # ---- coherence weights w + stats -------------------------------
            cmx = small.tile([1, 1], f32)
            nc.vector.tensor_reduce(out=cmx, in_=coh, axis=mybir.AxisListType.X, op=OP.max)
            cmn = small.tile([1, 1], f32)
            nc.vector.tensor_reduce(out=cmn, in_=coh, axis=mybir.AxisListType.X, op=OP.min)
            rng = small.tile([1, 1], f32)
            # (cmax + 1e-8) - cmin
            nc.vector.scalar_tensor_tensor(
                out=rng, in0=cmx, scalar=1e-8, in1=cmn, op0=OP.add, op1=OP.subtract
            )
            rcp = small.tile([1, 1], f32)
            nc.vector.reciprocal(out=rcp, in_=rng)
            w_sb = persist.tile([1, HW_N], f32)
            nc.vector.tensor_scalar(
                out=w_sb, in0=coh, scalar1=cmn, scalar2=rcp, op0=OP.subtract, op1=OP.mult
            )

            # stats: s1 = sum(w), s2 = sum(w^2), wmx = max(w)
            s1 = small.tile([1, 1], f32)
            nc.vector.tensor_reduce(out=s1, in_=w_sb, axis=mybir.AxisListType.X, op=OP.add)
            w2 = small.tile([1, HW_N], f32, tag="wrow")
            nc.vector.tensor_mul(out=w2, in0=w_sb, in1=w_sb)
            s2 = small.tile([1, 1], f32)
            nc.vector.tensor_reduce(out=s2, in_=w2, axis=mybir.AxisListType.X, op=OP.add)
            wmx = small.tile([1, 1], f32)
            nc.vector.tensor_reduce(out=wmx, in_=w_sb, axis=mybir.AxisListType.X, op=OP.max)
            # std = sqrt(s2/N - (s1/N)^2) via exp(0.5*ln(var))
            m1 = small.tile([1, 1], f32)
            nc.vector.tensor_scalar_mul(out=m1, in0=s1, scalar1=1.0 / HW_N)
            msq = small.tile([1, 1], f32)
            nc.vector.tensor_mul(out=msq, in0=m1, in1=m1)
            var = small.tile([1, 1], f32)
            nc.vector.scalar_tensor_tensor(
                out=var, in0=s2, scalar=1.0 / HW_N, in1=msq, op0=OP.mult, op1=OP.subtract
            )
            lnv = small.tile([1, 1], f32)
            nc.scalar.activation(out=lnv, in_=var, func=AF.Ln)
            std = small.tile([1, 1], f32)
            nc.scalar.activation(out=std, in_=lnv, func=AF.Exp, scale=0.5)
            # stats row [1,3] = [s1, std, max]  (g1st row 0 is pre-scaled by 1/N)
            strow = small.tile([1, 3], f32)
            nc.vector.tensor_copy(out=strow[:, 0:1], in_=s1)
            nc.vector.tensor_copy(out=strow[:, 1:2], in_=std)
            nc.vector.tensor_copy(out=strow[:, 2:3], in_=wmx)
            ps_st = ps_pv.tile([C, NCHUNK], f32, tag="pv")
            nc.tensor.matmul(out=ps_st[0:3, 0:1], lhsT=strow, rhs=one1,
                             start=True, stop=True, tile_position=(0, 0))
            stcol = small.tile([3, 1], f32)
            nc.vector.tensor_copy(out=stcol, in_=ps_st[0:3, 0:1])

            # ---- gate MLP ---------------------------------------------------
            vvs = small.tile([C, 1], f32, tag="col")
            nc.vector.tensor_reduce(out=vvs, in_=xvv, axis=mybir.AxisListType.X, op=OP.add)
            vhs = small.tile([C, 1], f32, tag="col")
            nc.vector.tensor_reduce(out=vhs, in_=xvh, axis=mybir.AxisListType.X, op=OP.add)

            ps_g = ps_pv.tile([C, NCHUNK], f32, tag="pv")
            h1p = ps_g[0:64, 0:1]
            nc.tensor.matmul(out=h1p, lhsT=g1vv, rhs=vvs, start=True, stop=False,
                             tile_position=(0, 0))
            nc.tensor.matmul(out=h1p, lhsT=g1vh, rhs=vhs, start=False, stop=False,
                             tile_position=(0, 0))
            nc.tensor.matmul(out=h1p, lhsT=g1st, rhs=stcol, start=False, stop=False,
                             tile_position=(0, 0))
            nc.tensor.matmul(out=h1p, lhsT=g1b, rhs=one1, start=False, stop=True,
                             tile_position=(0, 0))
            h1 = small.tile([64, 1], f32, tag="col")
            nc.scalar.activation(out=h1, in_=h1p, func=AF.Relu)

            ps_g2 = ps_pv.tile([C, NCHUNK], f32, tag="pv")
            h2p = ps_g2[0:32, 0:1]
            nc.tensor.matmul(out=h2p, lhsT=g2w, rhs=h1, start=True, stop=False,
                             tile_position=(0, 0))
            nc.tensor.matmul(out=h2p, lhsT=g2b, rhs=one1, start=False, stop=True,
                             tile_position=(0, 0))
            h2 = small.tile([32, 1], f32, tag="col")
            nc.scalar.activation(out=h2, in_=h2p, func=AF.Relu)

            ps_g3 = ps_pv.tile([C, NCHUNK], f32, tag="pv")
            zp = ps_g3[0:1, 0:1]
            nc.tensor.matmul(out=zp, lhsT=g3w, rhs=h2, start=True, stop=False,
                             tile_position=(0, 0))
            nc.tensor.matmul(out=zp, lhsT=g3b, rhs=one1, start=False, stop=True,
                             tile_position=(0, 0))
            # g = sigmoid(z) = 1/(1+exp(-z))
            ez = small.tile([1, 1], f32)
            nc.scalar.activation(out=ez, in_=zp, func=AF.Exp, scale=-1.0)
            gden = small.tile([1, 1], f32)
            nc.vector.tensor_scalar_add(out=gden, in0=ez, scalar1=1.0)
            gsc = small.tile([1, 1], f32)
            nc.vector.reciprocal(out=gsc, in_=gden)
            # broadcast g to all partitions
            ps_gb = ps_pv.tile([C, NCHUNK], f32, tag="pv")
            nc.tensor.matmul(out=ps_gb[:, 0:1], lhsT=ones, rhs=gsc,
                             start=True, stop=True, tile_position=(0, 0))
            g_col = persist.tile([C, 1], f32)
            nc.vector.tensor_copy(out=g_col, in_=ps_gb[:, 0:1])
            # c = (1-g)/g = 1/g - 1  (scalar, partition 0); wc_row = c*w
            rg1 = small.tile([1, 1], f32)
            nc.vector.reciprocal(out=rg1, in_=gsc)
            c1 = small.tile([1, 1], f32)
            nc.vector.tensor_scalar_add(out=c1, in0=rg1, scalar1=-1.0)
            wc_row = persist.tile([1, HW_N], f32r)
            nc.vector.tensor_scalar_mul(out=wc_row, in0=w_sb, scalar1=c1)
            w_r = persist.tile([1, HW_N], f32r)
            nc.vector.tensor_copy(out=w_r, in_=w_sb)

            # ---- V' projection (token-major, 17-col head groups + ones) ----
            # vp[p, mc, h, 0] = 1;  vp[p, mc, h, 1:17] = V_seq[mc*128+p, 16h:16h+16]
            # (ones first so the PV denominator row lands on the 32-aligned
            #  strip base - engine APs must start at partition 0/32/64/96)
            vp = persist.tile([C, MT, NH, HD + 1], f32r)
            # memset can't write f32r; broadcast-copy rounds f32 -> f32r
            nc.vector.tensor_copy(out=vp[:, :, :, 0:1],
                                  in_=ones_col.to_broadcast([C, MT, NH, 1]))
            for gp in range(2):
                ps_v = ps_pv.tile([C, NCHUNK], f32, tag="pv")
                for i in range(4):
                    mc = 4 * gp + i
                    nc.tensor.matmul(
                        out=ps_v[:, i * C : (i + 1) * C],
                        lhsT=xvh[:, mc * C : (mc + 1) * C],
                        rhs=wv,
                        start=True, stop=True, tile_position=(0, 0),
                    )
                for i in range(4):
                    mc = 4 * gp + i
                    nc.vector.tensor_add(
                        out=vp[:, mc, :, 1 : HD + 1],
                        in0=ps_v[:, i * C : (i + 1) * C].rearrange(
                            "p (h d) -> p h d", h=NH
                        ),
                        in1=bvb.rearrange("p (h d) -> p h d", h=NH),
                    )

            # ---- Q^T / K^T projections (pair-padded head layout) -----------
            qt = qkbuf.tile([C, NPAIR, HW_N], f32r)
            kt = qkbuf.tile([C, NPAIR, HW_N], f32r)
            for t in range(NPAIR):
                ps_q = ps_s.tile([C, HW_N], f32, tag="s")
                for ncb in range(2):
                    sl = slice(ncb * NCHUNK, (ncb + 1) * NCHUNK)
                    nc.tensor.matmul(out=ps_q[:, sl], lhsT=wq[:, t, :], rhs=xvv[:, sl],
                                     start=True, stop=True, tile_position=(0, 0))
                nc.vector.tensor_scalar_add(out=qt[:, t, :], in0=ps_q, scalar1=bq[:, t : t + 1])
            for t in range(NPAIR):
                ps_k = ps_s.tile([C, HW_N], f32, tag="s")
                for ncb in range(2):
                    sl = slice(ncb * NCHUNK, (ncb + 1) * NCHUNK)
                    nc.tensor.matmul(out=ps_k[:, sl], lhsT=wk[:, t, :], rhs=xvh[:, sl],
                                     start=True, stop=True, tile_position=(0, 0))
                nc.vector.tensor_scalar_add(out=kt[:, t, :], in0=ps_k, scalar1=bk[:, t : t + 1])

            # The blended scores are computed as one K=17 contraction:
            # row 16 of each strip of kt holds w, of qt holds c*w, so
            # S^T + c*w w^T comes out of a single matmul. Engine writes
            # can't target partition 16/80 (alignment), but DMA can.
            for t in range(NPAIR):
                for s_ in (HD, 64 + HD):
                    nc.sync.dma_start(out=kt[s_ : s_ + 1, t, :], in_=w_r)
                    nc.sync.dma_start(out=qt[s_ : s_ + 1, t, :], in_=wc_row)

            # ---- attention: S^T -> exp -> PV, flash-style ------------------
            aoutA = persist.tile([C, HW_N], f32r)
            aoutB = persist.tile([C, HW_N], f32r)
            nc.vector.tensor_copy(out=aoutA, in_=zero_col.to_broadcast([C, HW_N]))
            nc.vector.tensor_copy(out=aoutB, in_=zero_col.to_broadcast([C, HW_N]))
            for t in range(NPAIR):
                # per-(head,nchunk) PV psum tiles at partition base 0:
                # f32r matmuls require dst base partition 0.
                pv_tiles = {
                    (si, ncb): ps_pv.tile(
                        [C, NCHUNK], f32, tag="pv", name=f"pv_{t}_{si}_{ncb}"
                    )
                    for si in range(2)
                    for ncb in range(2)
                }
                for mt_i in range(MT):
                    msl = slice(mt_i * C, (mt_i + 1) * C)
                    es_tiles = []
                    for si, s in enumerate((0, 64)):
                        h = 2 * t + si
                        ps = ps_s.tile([C, HW_N], f32, tag="s")
                        for ncb in range(2):
                            sl = slice(ncb * NCHUNK, (ncb + 1) * NCHUNK)
                            nc.tensor.matmul(
                                out=ps[:, sl],
                                lhsT=kt[s : s + HD + 1, t, msl],
                                rhs=qt[s : s + HD + 1, t, sl],
                                start=True, stop=True, tile_position=(s, 0),
                            )
                        es = expp.tile([C, HW_N], f32r, tag="es")
                        nc.scalar.activation(out=es, in_=ps, func=AF.Exp, scale=g_col)
                        es_tiles.append((si, es))
                    for si, es in es_tiles:
                        h = 2 * t + si
                        for ncb in range(2):
                            sl = slice(ncb * NCHUNK, (ncb + 1) * NCHUNK)
                            nc.tensor.matmul(
                                out=pv_tiles[si, ncb][0 : HD + 1, :],
                                lhsT=vp[:, mt_i, h, :],
                                rhs=es[:, sl],
                                start=(mt_i == 0), stop=(mt_i == MT - 1),
                                tile_position=(0, 0),
                            )
                # normalize this pair's two heads
                for si in range(2):
                    h = 2 * t + si
                    j = h % 4
                    for ncb in range(2):
                        pvt = pv_tiles[si, ncb]
                        rd = rdp.tile([1, NCHUNK], f32, tag="rd")
                        nc.vector.reciprocal_approx_fast(out=rd, in_=pvt[0:1, :])
                        rdb = rdbp.tile([HD + 1, NCHUNK], f32, tag="rdb")
                        nc.gpsimd.partition_broadcast(rdb, rd)
                        # row 32j gets denom*recip(denom) ~= 1 junk; it hits a
                        # zero row of the padded Wo so it never contributes.
                        dst = aoutA if h < 4 else aoutB
                        nc.vector.tensor_mul(
                            out=dst[32 * j : 32 * j + HD + 1,
                                    ncb * NCHUNK : (ncb + 1) * NCHUNK],
                            in0=pvt[0 : HD + 1, :],
                            in1=rdb,
                        )

            # rounded copies of the padded output-projection weights
            woa_r = persist.tile([C, C], f32r)
            nc.vector.tensor_copy(out=woa_r, in_=woa)
            wob_r = persist.tile([C, C], f32r)
            nc.vector.tensor_copy(out=wob_r, in_=wob)

            # ---- output projection + bias + residual -----------------------
            ps_y = ps_s.tile([C, HW_N], f32, tag="s")
            for ncb in range(2):
                sl = slice(ncb * NCHUNK, (ncb + 1) * NCHUNK)
                nc.tensor.matmul(out=ps_y[:, sl], lhsT=woa_r, rhs=aoutA[:, sl],
                                 start=True, stop=False, tile_position=(0, 0))
                nc.tensor.matmul(out=ps_y[:, sl], lhsT=wob_r, rhs=aoutB[:, sl],
                                 start=False, stop=True, tile_position=(0, 0))
            y_sb = persist.tile([C, HW_N], f32)
            # (ps_y + bo) + x_vv
            nc.vector.scalar_tensor_tensor(
                out=y_sb, in0=ps_y, scalar=bo, in1=xvv, op0=OP.add, op1=OP.add
            )
            nc.sync.dma_start(out=y_d[:, :], in_=y_sb)

    nc.compile()
    _CACHE["nc"] = nc
    return nc


def make_in_maps(inputs: dict) -> list[dict]:
    """Host-side prep: shard over batch, pre-transpose/pad the small weights."""
    f32 = np.float32
    vv = np.ascontiguousarray(inputs["vv_features"], dtype=f32)
    vh = np.ascontiguousarray(inputs["vh_features"], dtype=f32)
    coh = np.ascontiguousarray(inputs["coherence_matrix"], dtype=f32)
    Wq = np.asarray(inputs["Wq"], f32)
    bq = np.asarray(inputs["bq"], f32)
    Wk = np.asarray(inputs["Wk"], f32)
    bk = np.asarray(inputs["bk"], f32)
    Wv = np.asarray(inputs["Wv"], f32)
    bv = np.asarray(inputs["bv"], f32)
    Wo = np.asarray(inputs["Wo"], f32)
    bo = np.asarray(inputs["bo"], f32)
    temp = float(np.asarray(inputs["temperature"], f32).reshape(-1)[0])
    g1w = np.asarray(inputs["g1w"], f32)
    g1b = np.asarray(inputs["g1b"], f32)
    g2w = np.asarray(inputs["g2w"], f32)
    g2b = np.asarray(inputs["g2b"], f32)
    g3w = np.asarray(inputs["g3w"], f32)
    g3b = np.asarray(inputs["g3b"], f32)

    def _wo_pad(Wo_, grp):
        # lhsT [c_in_padded=128, c_out=128]: strip j row d holds Wo column for
        # channel 16*(4*grp+j)+d; pad rows (d>=16) are zero.
        wp = np.zeros((C, C), f32)
        for j in range(4):
            ch0 = HD * (4 * grp + j)
            wp[32 * j + 1 : 32 * j + 1 + HD, :] = Wo_[:, ch0 : ch0 + HD].T
        return np.ascontiguousarray(wp)

    def pad_pair(Wt, bt):
        # -> lhsT tiles [C, NPAIR, C] flattened to [C, NPAIR*C]; bias [C, NPAIR]
        wpad = np.zeros((NPAIR, C, C), f32)   # [t, c_in, m]
        bpad = np.zeros((C, NPAIR), f32)
        for t in range(NPAIR):
            wpad[t, :, 0:HD] = Wt[2 * t * HD : (2 * t + 1) * HD, :].T
            wpad[t, :, 64 : 64 + HD] = Wt[(2 * t + 1) * HD : (2 * t + 2) * HD, :].T
            bpad[0:HD, t] = bt[2 * t * HD : (2 * t + 1) * HD]
            bpad[64 : 64 + HD, t] = bt[(2 * t + 1) * HD : (2 * t + 2) * HD]
        wflat = np.ascontiguousarray(wpad.transpose(1, 0, 2).reshape(C, NPAIR * C))
        return wflat, np.ascontiguousarray(bpad)

    wq_h, bq_h = pad_pair(Wq * temp, bq * temp)
    wk_h, bk_h = pad_pair(Wk, bk)

    shared = {
        "wq": wq_h, "bq": bq_h, "wk": wk_h, "bk": bk_h,
        "wv": np.ascontiguousarray(Wv.T), "bv": np.ascontiguousarray(np.tile(bv, (C, 1))),
        "woa": _wo_pad(Wo, 0), "wob": _wo_pad(Wo, 1),
        "bo": np.ascontiguousarray(bo[:, None]),
        "g1vv": np.ascontiguousarray(g1w[:, :C].T / HW_N),
        "g1vh": np.ascontiguousarray(g1w[:, C : 2 * C].T / HW_N),
        "g1st": np.ascontiguousarray(g1w[:, 2 * C : 2 * C + 3].T * np.array([1.0 / HW_N, 1.0, 1.0], f32)[:, None]),
        "g1b": np.ascontiguousarray(g1b[None, :]),
        "g2w": np.ascontiguousarray(g2w.T), "g2b": np.ascontiguousarray(g2b[None, :]),
        "g3w": np.ascontiguousarray(g3w.T), "g3b": np.ascontiguousarray(g3b[None, :]),
    }
    in_maps = []
    for b in range(B):
        m = dict(shared)
        m["x_vv"] = np.ascontiguousarray(vv[b].reshape(C, HW_N))
        m["x_vh"] = np.ascontiguousarray(vh[b].reshape(C, HW_N))
        m["coh"] = np.ascontiguousarray(coh[b].reshape(1, HW_N))
        in_maps.append(m)
    return in_maps


def kernel(**inputs) -> np.ndarray:
    nc = build_program()
    in_maps = make_in_maps(inputs)
    from concourse.bass_utils import run_bass_kernel_spmd

    res = run_bass_kernel_spmd(nc, in_maps, core_ids=list(range(NCORES)))
    out = np.stack([res.results[i]["y"].reshape(C, 32, 32) for i in range(B)])
    return np.ascontiguousarray(out.astype(np.float32))


# revision 11
# speedup vs baseline: 2.9753x; 1.0756x over previous
"""CoherenceGuidedAttention Trainium2 Bass kernel.

Sharding: data-parallel over batch B=8 -> one batch per NeuronCore (8 cores).
All parameters replicated; no collectives.

Per-core algorithm (flash-style, channel-major "layout B"):
  X_vv, X_vh       : [C=128, N=1024] SBUF (natural layout of [C,H,W] input)
  Q^T = temp*(Wq X_vv + bq), K^T = Wk X_vh + bk   (pair-padded head layout)
  V'  = (X_vh^T Wv + bv | ones)  token-major with per-head 17-col groups
  S^T(h) = K_h Q_h^T   computed per (head, m-tile) into PSUM [128,1024]
           + rank-1 accumulate  c*w w^T  (c=(1-g)/g) via K=1 matmuls
  expS = exp(g * S^T_blend)  on ACT directly from PSUM (scale = per-partition g)
  PV   = V'_h^T expS  accumulated over m-chunks -> [17, 512] (row 16 = denom)
  Aout^T[h] = PV[0:16] * (1/denom)  (fast reciprocal + partition broadcast)
  Y^T = Wo Aout^T + bo + X_vv  -> output [C, N]

Head layout for PE row-tiling: Q^T/K^T stored as 4 pair-tiles [128, 1024];
pair t holds head 2t at partitions 0:16 and head 2t+1 at partitions 64:80,
so the two heads' QK^T matmuls run concurrently in PE row-strips 0 and 64.
"""

import sys

if "/opt/trn_rl_repo" not in sys.path:
    sys.path.insert(0, "/opt/trn_rl_repo")

import numpy as np

B = 8
C = 128
HW_N = 1024  # H*W tokens
NH = 8       # heads
HD = 16      # head dim
NCORES = 8
NPAIR = 4    # head pairs
MT = 8       # m tiles of 128 tokens
NCHUNK = 512  # fp32 moving-operand max

_CACHE: dict = {}


def build_program():
    """Build (once) the SPMD Bass program for one core."""
    if "nc" in _CACHE:
        return _CACHE["nc"]

    from concourse import bacc
    import concourse.mybir as mybir
    import concourse.tile as tile

    f32 = mybir.dt.float32
    f32r = mybir.dt.float32r
    bf16 = mybir.dt.bfloat16
    AF = mybir.ActivationFunctionType
    OP = mybir.AluOpType

    nc = bacc.Bacc("TRN2", target_bir_lowering=False, debug=False)

    # ---- DRAM I/O ----------------------------------------------------------
    x_vv_d = nc.dram_tensor("x_vv", [C, HW_N], f32, kind="ExternalInput")
    x_vh_d = nc.dram_tensor("x_vh", [C, HW_N], f32, kind="ExternalInput")
    coh_d = nc.dram_tensor("coh", [1, HW_N], f32, kind="ExternalInput")
    wq_d = nc.dram_tensor("wq", [C, NPAIR * C], f32, kind="ExternalInput")
    wk_d = nc.dram_tensor("wk", [C, NPAIR * C], f32, kind="ExternalInput")
    bq_d = nc.dram_tensor("bq", [C, NPAIR], f32, kind="ExternalInput")
    bk_d = nc.dram_tensor("bk", [C, NPAIR], f32, kind="ExternalInput")
    wv_d = nc.dram_tensor("wv", [C, C], f32, kind="ExternalInput")
    bv_d = nc.dram_tensor("bv", [C, C], f32, kind="ExternalInput")
    woa_d = nc.dram_tensor("woa", [C, C], f32, kind="ExternalInput")
    wob_d = nc.dram_tensor("wob", [C, C], f32, kind="ExternalInput")
    bo_d = nc.dram_tensor("bo", [C, 1], f32, kind="ExternalInput")
    g1vv_d = nc.dram_tensor("g1vv", [C, 64], f32, kind="ExternalInput")
    g1vh_d = nc.dram_tensor("g1vh", [C, 64], f32, kind="ExternalInput")
    g1st_d = nc.dram_tensor("g1st", [3, 64], f32, kind="ExternalInput")
    g1b_d = nc.dram_tensor("g1b", [1, 64], f32, kind="ExternalInput")
    g2w_d = nc.dram_tensor("g2w", [64, 32], f32, kind="ExternalInput")
    g2b_d = nc.dram_tensor("g2b", [1, 32], f32, kind="ExternalInput")
    g3w_d = nc.dram_tensor("g3w", [32, 1], f32, kind="ExternalInput")
    g3b_d = nc.dram_tensor("g3b", [1, 1], f32, kind="ExternalInput")
    y_d = nc.dram_tensor("y", [C, HW_N], f32, kind="ExternalOutput")

    with tile.TileContext(nc) as tc:
        with (
            tc.tile_pool(name="persist", bufs=1) as persist,
            tc.tile_pool(name="qkbuf", bufs=1) as qkbuf,
            tc.tile_pool(name="expp", bufs=6) as expp,
            tc.tile_pool(name="small", bufs=8) as small,
            tc.tile_pool(name="rdp", bufs=4) as rdp,
            tc.tile_pool(name="rdbp", bufs=4) as rdbp,
            tc.tile_pool(name="ps_s", bufs=2, space="PSUM") as ps_s,
            tc.tile_pool(name="ps_pv", bufs=4, space="PSUM") as ps_pv,
        ):
            # ---- load inputs ------------------------------------------------
            xvv = persist.tile([C, HW_N], f32)
            xvh = persist.tile([C, HW_N], f32)
            coh = persist.tile([1, HW_N], f32)
            wq = persist.tile([C, NPAIR, C], f32)
            wk = persist.tile([C, NPAIR, C], f32)
            bq = persist.tile([C, NPAIR], f32)
            bk = persist.tile([C, NPAIR], f32)
            wv = persist.tile([C, C], f32)
            bvb = persist.tile([C, C], f32)
            woa = persist.tile([C, C], f32)
            wob = persist.tile([C, C], f32)
            bo = persist.tile([C, 1], f32)
            g1vv = persist.tile([C, 64], f32)
            g1vh = persist.tile([C, 64], f32)
            g1st = persist.tile([3, 64], f32)
            g1b = persist.tile([1, 64], f32)
            g2w = persist.tile([64, 32], f32)
            g2b = persist.tile([1, 32], f32)
            g3w = persist.tile([32, 1], f32)
            g3b = persist.tile([1, 1], f32)

            nc.sync.dma_start(out=xvv, in_=x_vv_d[:, :])
            nc.sync.dma_start(out=xvh, in_=x_vh_d[:, :])
            nc.sync.dma_start(out=coh, in_=coh_d[:, :])
            nc.sync.dma_start(out=wq, in_=wq_d.ap().rearrange("p (t m) -> p t m", t=NPAIR))
            nc.sync.dma_start(out=wk, in_=wk_d.ap().rearrange("p (t m) -> p t m", t=NPAIR))
            nc.sync.dma_start(out=bq, in_=bq_d[:, :])
            nc.sync.dma_start(out=bk, in_=bk_d[:, :])
            nc.sync.dma_start(out=wv, in_=wv_d[:, :])
            nc.sync.dma_start(out=bvb, in_=bv_d[:, :])
            nc.sync.dma_start(out=woa, in_=woa_d[:, :])
            nc.sync.dma_start(out=wob, in_=wob_d[:, :])
            nc.sync.dma_start(out=bo, in_=bo_d[:, :])
            nc.sync.dma_start(out=g1vv, in_=g1vv_d[:, :])
            nc.sync.dma_start(out=g1vh, in_=g1vh_d[:, :])
            nc.sync.dma_start(out=g1st, in_=g1st_d[:, :])
            nc.sync.dma_start(out=g1b, in_=g1b_d[:, :])
            nc.sync.dma_start(out=g2w, in_=g2w_d[:, :])
            nc.sync.dma_start(out=g2b, in_=g2b_d[:, :])
            nc.sync.dma_start(out=g3w, in_=g3w_d[:, :])
            nc.sync.dma_start(out=g3b, in_=g3b_d[:, :])

            ones = persist.tile([1, C], f32)
            nc.vector.memset(ones, 1.0)
            one1 = ones[:, 0:1]
            ones_col = persist.tile([C, 1], f32)
            nc.vector.memset(ones_col, 1.0)
            zero_col = persist.tile([C, 1], f32)
            nc.vector.memset(zero_col, 0.0)

            # rounded f32r views for the f32r projection matmuls
            xvv_r = persist.tile([C, HW_N], f32r)
            nc.vector.tensor_copy(out=xvv_r, in_=xvv)
            xvh_r = persist.tile([C, HW_N], f32r)
            nc.vector.tensor_copy(out=xvh_r, in_=xvh)
            wq_r = persist.tile([C, NPAIR, C], f32r)
            nc.vector.tensor_copy(out=wq_r, in_=wq)
            wk_r = persist.tile([C, NPAIR, C], f32r)
            nc.vector.tensor_copy(out=wk_r, in_=wk)
            wv_r = persist.tile([C, C], f32r)
            nc.vector.tensor_copy(out=wv_r, in_=wv)

            # ---- coherence weights w + stats -------------------------------
            cmx = small.tile([1, 1], f32)
            nc.vector.tensor_reduce(out=cmx, in_=coh, axis=mybir.AxisListType.X, op=OP.max)
            cmn = small.tile([1, 1], f32)
            nc.vector.tensor_reduce(out=cmn, in_=coh, axis=mybir.AxisListType.X, op=OP.min)
            rng = small.tile([1, 1], f32)
            # (cmax + 1e-8) - cmin
            nc.vector.scalar_tensor_tensor(
                out=rng, in0=cmx, scalar=1e-8, in1=cmn, op0=OP.add, op1=OP.subtract
            )
            rcp = small.tile([1, 1], f32)
            nc.vector.reciprocal(out=rcp, in_=rng)
            w_sb = persist.tile([1, HW_N], f32)
            nc.vector.tensor_scalar(
                out=w_sb, in0=coh, scalar1=cmn, scalar2=rcp, op0=OP.subtract, op1=OP.mult
            )

            # stats: s1 = sum(w), s2 = sum(w^2), wmx = max(w)
            s1 = small.tile([1, 1], f32)
            nc.vector.tensor_reduce(out=s1, in_=w_sb, axis=mybir.AxisListType.X, op=OP.add)
            w2 = small.tile([1, HW_N], f32, tag="wrow")
            nc.vector.tensor_mul(out=w2, in0=w_sb, in1=w_sb)
            s2 = small.tile([1, 1], f32)
            nc.vector.tensor_reduce(out=s2, in_=w2, axis=mybir.AxisListType.X, op=OP.add)
            wmx = small.tile([1, 1], f32)
            nc.vector.tensor_reduce(out=wmx, in_=w_sb, axis=mybir.AxisListType.X, op=OP.max)
            # std = sqrt(s2/N - (s1/N)^2) via exp(0.5*ln(var))
            m1 = small.tile([1, 1], f32)
            nc.vector.tensor_scalar_mul(out=m1, in0=s1, scalar1=1.0 / HW_N)
            msq = small.tile([1, 1], f32)
            nc.vector.tensor_mul(out=msq, in0=m1, in1=m1)
            var = small.tile([1, 1], f32)
            nc.vector.scalar_tensor_tensor(
                out=var, in0=s2, scalar=1.0 / HW_N, in1=msq, op0=OP.mult, op1=OP.subtract
            )
            lnv = small.tile([1, 1], f32)
            nc.scalar.activation(out=lnv, in_=var, func=AF.Ln)
            std = small.tile([1, 1], f32)
            nc.scalar.activation(out=std, in_=lnv, func=AF.Exp, scale=0.5)
            # stats row [1,3] = [s1, std, max]  (g1st row 0 is pre-scaled by 1/N)
            strow = small.tile([1, 3], f32)
            nc.vector.tensor_copy(out=strow[:, 0:1], in_=s1)
            nc.vector.tensor_copy(out=strow[:, 1:2], in_=std)
            nc.vector.tensor_copy(out=strow[:, 2:3], in_=wmx)
            ps_st = ps_pv.tile([C, NCHUNK], f32, tag="pv")
            nc.tensor.matmul(out=ps_st[0:3, 0:1], lhsT=strow, rhs=one1,
                             start=True, stop=True, tile_position=(0, 0))
            stcol = small.tile([3, 1], f32)
            nc.vector.tensor_copy(out=stcol, in_=ps_st[0:3, 0:1])

            # ---- gate MLP ---------------------------------------------------
            vvs = small.tile([C, 1], f32, tag="col")
            nc.vector.tensor_reduce(out=vvs, in_=xvv, axis=mybir.AxisListType.X, op=OP.add)
            vhs = small.tile([C, 1], f32, tag="col")
            nc.vector.tensor_reduce(out=vhs, in_=xvh, axis=mybir.AxisListType.X, op=OP.add)

            ps_g = ps_pv.tile([C, NCHUNK], f32, tag="pv")
            h1p = ps_g[0:64, 0:1]
            nc.tensor.matmul(out=h1p, lhsT=g1vv, rhs=vvs, start=True, stop=False,
                             tile_position=(0, 0))
            nc.tensor.matmul(out=h1p, lhsT=g1vh, rhs=vhs, start=False, stop=False,
                             tile_position=(0, 0))
            nc.tensor.matmul(out=h1p, lhsT=g1st, rhs=stcol, start=False, stop=False,
                             tile_position=(0, 0))
            nc.tensor.matmul(out=h1p, lhsT=g1b, rhs=one1, start=False, stop=True,
                             tile_position=(0, 0))
            h1 = small.tile([64, 1], f32, tag="col")
            nc.scalar.activation(out=h1, in_=h1p, func=AF.Relu)

            ps_g2 = ps_pv.tile([C, NCHUNK], f32, tag="pv")
            h2p = ps_g2[0:32, 0:1]
            nc.tensor.matmul(out=h2p, lhsT=g2w, rhs=h1, start=True, stop=False,
                             tile_position=(0, 0))
            nc.tensor.matmul(out=h2p, lhsT=g2b, rhs=one1, start=False, stop=True,
                             tile_position=(0, 0))
            h2 = small.tile([32, 1], f32, tag="col")
            nc.scalar.activation(out=h2, in_=h2p, func=AF.Relu)

            ps_g3 = ps_pv.tile([C, NCHUNK], f32, tag="pv")
            zp = ps_g3[0:1, 0:1]
            nc.tensor.matmul(out=zp, lhsT=g3w, rhs=h2, start=True, stop=False,
                             tile_position=(0, 0))
            nc.tensor.matmul(out=zp, lhsT=g3b, rhs=one1, start=False, stop=True,
                             tile_position=(0, 0))
            # g = sigmoid(z) = 1/(1+exp(-z))
            ez = small.tile([1, 1], f32)
            nc.scalar.activation(out=ez, in_=zp, func=AF.Exp, scale=-1.0)
            gden = small.tile([1, 1], f32)
            nc.vector.tensor_scalar_add(out=gden, in0=ez, scalar1=1.0)
            gsc = small.tile([1, 1], f32)
            nc.vector.reciprocal(out=gsc, in_=gden)
            # broadcast g to all partitions
            ps_gb = ps_pv.tile([C, NCHUNK], f32, tag="pv")
            nc.tensor.matmul(out=ps_gb[:, 0:1], lhsT=ones, rhs=gsc,
                             start=True, stop=True, tile_position=(0, 0))
            g_col = persist.tile([C, 1], f32)
            nc.vector.tensor_copy(out=g_col, in_=ps_gb[:, 0:1])
            # c = (1-g)/g = 1/g - 1  (scalar, partition 0); wc_row = c*w
            rg1 = small.tile([1, 1], f32)
            nc.vector.reciprocal(out=rg1, in_=gsc)
            c1 = small.tile([1, 1], f32)
            nc.vector.tensor_scalar_add(out=c1, in0=rg1, scalar1=-1.0)
            wc_row = persist.tile([1, HW_N], bf16)
            nc.vector.tensor_scalar_mul(out=wc_row, in0=w_sb, scalar1=c1)
            w_r = persist.tile([1, HW_N], bf16)
            nc.vector.tensor_copy(out=w_r, in_=w_sb)

            # ---- V' projection (token-major, 17-col head groups + ones) ----
            # vp[p, mc, h, 0] = 1;  vp[p, mc, h, 1:17] = V_seq[mc*128+p, 16h:16h+16]
            # (ones first so the PV denominator row lands on the 32-aligned
            #  strip base - engine APs must start at partition 0/32/64/96)
            vp = persist.tile([C, MT, NH, HD + 1], bf16)
            # memset can't write f32r; broadcast-copy rounds f32 -> f32r
            nc.vector.tensor_copy(out=vp[:, :, :, 0:1],
                                  in_=ones_col.to_broadcast([C, MT, NH, 1]))
            for gp in range(2):
                ps_v = ps_pv.tile([C, NCHUNK], f32, tag="pv")
                for i in range(4):
                    mc = 4 * gp + i
                    nc.tensor.matmul(
                        out=ps_v[:, i * C : (i + 1) * C],
                        lhsT=xvh_r[:, mc * C : (mc + 1) * C],
                        rhs=wv_r,
                        start=True, stop=True, tile_position=(0, 0),
                    )
                for i in range(4):
                    mc = 4 * gp + i
                    nc.vector.tensor_add(
                        out=vp[:, mc, :, 1 : HD + 1],
                        in0=ps_v[:, i * C : (i + 1) * C].rearrange(
                            "p (h d) -> p h d", h=NH
                        ),
                        in1=bvb.rearrange("p (h d) -> p h d", h=NH),
                    )

            # ---- Q^T / K^T projections (pair-padded head layout) -----------
            qt = qkbuf.tile([C, NPAIR, HW_N], bf16)
            kt = qkbuf.tile([C, NPAIR, HW_N], bf16)
            for t in range(NPAIR):
                ps_q = ps_s.tile([C, HW_N], f32, tag="s")
                for ncb in range(2):
                    sl = slice(ncb * NCHUNK, (ncb + 1) * NCHUNK)
                    nc.tensor.matmul(out=ps_q[:, sl], lhsT=wq_r[:, t, :],
                                     rhs=xvv_r[:, sl],
                                     start=True, stop=True, tile_position=(0, 0))
                nc.vector.tensor_scalar_add(out=qt[:, t, :], in0=ps_q, scalar1=bq[:, t : t + 1])
            for t in range(NPAIR):
                ps_k = ps_s.tile([C, HW_N], f32, tag="s")
                for ncb in range(2):
                    sl = slice(ncb * NCHUNK, (ncb + 1) * NCHUNK)
                    nc.tensor.matmul(out=ps_k[:, sl], lhsT=wk_r[:, t, :],
                                     rhs=xvh_r[:, sl],
                                     start=True, stop=True, tile_position=(0, 0))
                nc.vector.tensor_scalar_add(out=kt[:, t, :], in0=ps_k, scalar1=bk[:, t : t + 1])

            # The blended scores are computed as one K=17 contraction:
            # row 16 of each strip of kt holds w, of qt holds c*w, so
            # S^T + c*w w^T comes out of a single matmul. Engine writes
            # can't target partition 16/80 (alignment), but DMA can.
            for t in range(NPAIR):
                for s_ in (HD, 64 + HD):
                    nc.sync.dma_start(out=kt[s_ : s_ + 1, t, :], in_=w_r)
                    nc.sync.dma_start(out=qt[s_ : s_ + 1, t, :], in_=wc_row)

            # ---- attention: S^T -> exp -> PV, flash-style ------------------
            aoutA = persist.tile([C, HW_N], bf16)
            aoutB = persist.tile([C, HW_N], bf16)
            nc.vector.tensor_copy(out=aoutA, in_=zero_col.to_broadcast([C, HW_N]))
            nc.vector.tensor_copy(out=aoutB, in_=zero_col.to_broadcast([C, HW_N]))
            for t in range(NPAIR):
                # per-(head,nchunk) PV psum tiles at partition base 0:
                # f32r matmuls require dst base partition 0.
                pv_tiles = {
                    (si, ncb): ps_pv.tile(
                        [C, NCHUNK], f32, tag="pv", name=f"pv_{t}_{si}_{ncb}"
                    )
                    for si in range(2)
                    for ncb in range(2)
                }
                for mt_i in range(MT):
                    msl = slice(mt_i * C, (mt_i + 1) * C)
                    es_tiles = []
                    for si, s in enumerate((0, 64)):
                        h = 2 * t + si
                        ps = ps_s.tile([C, HW_N], f32, tag="s")
                        for ncb in range(2):
                            sl = slice(ncb * NCHUNK, (ncb + 1) * NCHUNK)
                            nc.tensor.matmul(
                                out=ps[:, sl],
                                lhsT=kt[s : s + HD + 1, t, msl],
                                rhs=qt[s : s + HD + 1, t, sl],
                                start=True, stop=True, tile_position=(s, 0),
                            )
                        es = expp.tile([C, HW_N], bf16, tag="es")
                        nc.scalar.activation(out=es, in_=ps, func=AF.Exp, scale=g_col)
                        es_tiles.append((si, es))
                    for si, es in es_tiles:
                        h = 2 * t + si
                        for ncb in range(2):
                            sl = slice(ncb * NCHUNK, (ncb + 1) * NCHUNK)
                            nc.tensor.matmul(
                                out=pv_tiles[si, ncb][0 : HD + 1, :],
                                lhsT=vp[:, mt_i, h, :],
                                rhs=es[:, sl],
                                start=(mt_i == 0), stop=(mt_i == MT - 1),
                                tile_position=(0, 0),
                            )
                # normalize this pair's two heads
                for si in range(2):
                    h = 2 * t + si
                    j = h % 4
                    for ncb in range(2):
                        pvt = pv_tiles[si, ncb]
                        rd = rdp.tile([1, NCHUNK], f32, tag="rd")
                        nc.vector.reciprocal_approx_fast(out=rd, in_=pvt[0:1, :])
                        rdb = rdbp.tile([HD + 1, NCHUNK], f32, tag="rdb")
                        nc.gpsimd.partition_broadcast(rdb, rd)
                        # row 32j gets denom*recip(denom) ~= 1 junk; it hits a
                        # zero row of the padded Wo so it never contributes.
                        dst = aoutA if h < 4 else aoutB
                        nc.vector.tensor_mul(
                            out=dst[32 * j : 32 * j + HD + 1,
                                    ncb * NCHUNK : (ncb + 1) * NCHUNK],
                            in0=pvt[0 : HD + 1, :],
                            in1=rdb,
                        )

            # rounded copies of the padded output-projection weights
            woa_r = persist.tile([C, C], bf16)
            nc.vector.tensor_copy(out=woa_r, in_=woa)
            wob_r = persist.tile([C, C], bf16)
            nc.vector.tensor_copy(out=wob_r, in_=wob)

            # ---- output projection + bias + residual -----------------------
            ps_y = ps_s.tile([C, HW_N], f32, tag="s")
            for ncb in range(2):
                sl = slice(ncb * NCHUNK, (ncb + 1) * NCHUNK)
                nc.tensor.matmul(out=ps_y[:, sl], lhsT=woa_r, rhs=aoutA[:, sl],
                                 start=True, stop=False, tile_position=(0, 0))
                nc.tensor.matmul(out=ps_y[:, sl], lhsT=wob_r, rhs=aoutB[:, sl],
                                 start=False, stop=True, tile_position=(0, 0))
            y_sb = persist.tile([C, HW_N], f32)
            # (ps_y + bo) + x_vv
            nc.vector.scalar_tensor_tensor(
                out=y_sb, in0=ps_y, scalar=bo, in1=xvv, op0=OP.add, op1=OP.add
            )
            nc.sync.dma_start(out=y_d[:, :], in_=y_sb)

    nc.compile()
    _CACHE["nc"] = nc
    return nc


def make_in_maps(inputs: dict) -> list[dict]:
    """Host-side prep: shard over batch, pre-transpose/pad the small weights."""
    f32 = np.float32
    vv = np.ascontiguousarray(inputs["vv_features"], dtype=f32)
    vh = np.ascontiguousarray(inputs["vh_features"], dtype=f32)
    coh = np.ascontiguousarray(inputs["coherence_matrix"], dtype=f32)
    Wq = np.asarray(inputs["Wq"], f32)
    bq = np.asarray(inputs["bq"], f32)
    Wk = np.asarray(inputs["Wk"], f32)
    bk = np.asarray(inputs["bk"], f32)
    Wv = np.asarray(inputs["Wv"], f32)
    bv = np.asarray(inputs["bv"], f32)
    Wo = np.asarray(inputs["Wo"], f32)
    bo = np.asarray(inputs["bo"], f32)
    temp = float(np.asarray(inputs["temperature"], f32).reshape(-1)[0])
    g1w = np.asarray(inputs["g1w"], f32)
    g1b = np.asarray(inputs["g1b"], f32)
    g2w = np.asarray(inputs["g2w"], f32)
    g2b = np.asarray(inputs["g2b"], f32)
    g3w = np.asarray(inputs["g3w"], f32)
    g3b = np.asarray(inputs["g3b"], f32)

    def _wo_pad(Wo_, grp):
        # lhsT [c_in_padded=128, c_out=128]: strip j row d holds Wo column for
        # channel 16*(4*grp+j)+d; pad rows (d>=16) are zero.
        wp = np.zeros((C, C), f32)
        for j in range(4):
            ch0 = HD * (4 * grp + j)
            wp[32 * j + 1 : 32 * j + 1 + HD, :] = Wo_[:, ch0 : ch0 + HD].T
        return np.ascontiguousarray(wp)

    def pad_pair(Wt, bt):
        # -> lhsT tiles [C, NPAIR, C] flattened to [C, NPAIR*C]; bias [C, NPAIR]
        wpad = np.zeros((NPAIR, C, C), f32)   # [t, c_in, m]
        bpad = np.zeros((C, NPAIR), f32)
        for t in range(NPAIR):
            wpad[t, :, 0:HD] = Wt[2 * t * HD : (2 * t + 1) * HD, :].T
            wpad[t, :, 64 : 64 + HD] = Wt[(2 * t + 1) * HD : (2 * t + 2) * HD, :].T
            bpad[0:HD, t] = bt[2 * t * HD : (2 * t + 1) * HD]
            bpad[64 : 64 + HD, t] = bt[(2 * t + 1) * HD : (2 * t + 2) * HD]
        wflat = np.ascontiguousarray(wpad.transpose(1, 0, 2).reshape(C, NPAIR * C))
        return wflat, np.ascontiguousarray(bpad)

    wq_h, bq_h = pad_pair(Wq * temp, bq * temp)
    wk_h, bk_h = pad_pair(Wk, bk)

    shared = {
        "wq": wq_h, "bq": bq_h, "wk": wk_h, "bk": bk_h,
        "wv": np.ascontiguousarray(Wv.T), "bv": np.ascontiguousarray(np.tile(bv, (C, 1))),
        "woa": _wo_pad(Wo, 0), "wob": _wo_pad(Wo, 1),
        "bo": np.ascontiguousarray(bo[:, None]),
        "g1vv": np.ascontiguousarray(g1w[:, :C].T / HW_N),
        "g1vh": np.ascontiguousarray(g1w[:, C : 2 * C].T / HW_N),
        "g1st": np.ascontiguousarray(g1w[:, 2 * C : 2 * C + 3].T * np.array([1.0 / HW_N, 1.0, 1.0], f32)[:, None]),
        "g1b": np.ascontiguousarray(g1b[None, :]),
        "g2w": np.ascontiguousarray(g2w.T), "g2b": np.ascontiguousarray(g2b[None, :]),
        "g3w": np.ascontiguousarray(g3w.T), "g3b": np.ascontiguousarray(g3b[None, :]),
    }
    in_maps = []
    for b in range(B):
        m = dict(shared)
        m["x_vv"] = np.ascontiguousarray(vv[b].reshape(C, HW_N))
        m["x_vh"] = np.ascontiguousarray(vh[b].reshape(C, HW_N))
        m["coh"] = np.ascontiguousarray(coh[b].reshape(1, HW_N))
        in_maps.append(m)
    return in_maps


def kernel(**inputs) -> np.ndarray:
    nc = build_program()
    in_maps = make_in_maps(inputs)
    from concourse.bass_utils import run_bass_kernel_spmd

    res = run_bass_kernel_spmd(nc, in_maps, core_ids=list(range(NCORES)))
    out = np.stack([res.results[i]["y"].reshape(C, 32, 32) for i in range(B)])
    return np.ascontiguousarray(out.astype(np.float32))


# revision 12
# speedup vs baseline: 3.1316x; 1.0525x over previous
"""CoherenceGuidedAttention Trainium2 Bass kernel.

Sharding: data-parallel over batch B=8 -> one batch per NeuronCore (8 cores).
All parameters replicated; no collectives.

Per-core algorithm (flash-style, channel-major "layout B"):
  X_vv, X_vh       : [C=128, N=1024] SBUF (natural layout of [C,H,W] input)
  Q^T = temp*(Wq X_vv + bq), K^T = Wk X_vh + bk   (pair-padded head layout)
  V'  = (X_vh^T Wv + bv | ones)  token-major with per-head 17-col groups
  S^T(h) = K_h Q_h^T   computed per (head, m-tile) into PSUM [128,1024]
           + rank-1 accumulate  c*w w^T  (c=(1-g)/g) via K=1 matmuls
  expS = exp(g * S^T_blend)  on ACT directly from PSUM (scale = per-partition g)
  PV   = V'_h^T expS  accumulated over m-chunks -> [17, 512] (row 16 = denom)
  Aout^T[h] = PV[0:16] * (1/denom)  (fast reciprocal + partition broadcast)
  Y^T = Wo Aout^T + bo + X_vv  -> output [C, N]

Head layout for PE row-tiling: Q^T/K^T stored as 4 pair-tiles [128, 1024];
pair t holds head 2t at partitions 0:16 and head 2t+1 at partitions 64:80,
so the two heads' QK^T matmuls run concurrently in PE row-strips 0 and 64.
"""

import sys

if "/opt/trn_rl_repo" not in sys.path:
    sys.path.insert(0, "/opt/trn_rl_repo")

import numpy as np

B = 8
C = 128
HW_N = 1024  # H*W tokens
NH = 8       # heads
HD = 16      # head dim
NCORES = 8
NPAIR = 4    # head pairs
MT = 8       # m tiles of 128 tokens
NCHUNK = 512  # fp32 moving-operand max

_CACHE: dict = {}


def build_program():
    """Build (once) the SPMD Bass program for one core."""
    if "nc" in _CACHE:
        return _CACHE["nc"]

    from concourse import bacc
    import concourse.mybir as mybir
    import concourse.tile as tile

    f32 = mybir.dt.float32
    f32r = mybir.dt.float32r
    bf16 = mybir.dt.bfloat16
    AF = mybir.ActivationFunctionType
    OP = mybir.AluOpType

    nc = bacc.Bacc("TRN2", target_bir_lowering=False, debug=False)

    # ---- DRAM I/O ----------------------------------------------------------
    x_vv_d = nc.dram_tensor("x_vv", [C, HW_N], f32, kind="ExternalInput")
    x_vh_d = nc.dram_tensor("x_vh", [C, HW_N], f32, kind="ExternalInput")
    coh_d = nc.dram_tensor("coh", [1, HW_N], f32, kind="ExternalInput")
    wq_d = nc.dram_tensor("wq", [C, NPAIR * C], f32, kind="ExternalInput")
    wk_d = nc.dram_tensor("wk", [C, NPAIR * C], f32, kind="ExternalInput")
    bq_d = nc.dram_tensor("bq", [C, NPAIR], f32, kind="ExternalInput")
    bk_d = nc.dram_tensor("bk", [C, NPAIR], f32, kind="ExternalInput")
    wv_d = nc.dram_tensor("wv", [C, C], f32, kind="ExternalInput")
    bv_d = nc.dram_tensor("bv", [C, C], f32, kind="ExternalInput")
    woa_d = nc.dram_tensor("woa", [C, C], f32, kind="ExternalInput")
    wob_d = nc.dram_tensor("wob", [C, C], f32, kind="ExternalInput")
    bo_d = nc.dram_tensor("bo", [C, 1], f32, kind="ExternalInput")
    g1vv_d = nc.dram_tensor("g1vv", [C, 64], f32, kind="ExternalInput")
    g1vh_d = nc.dram_tensor("g1vh", [C, 64], f32, kind="ExternalInput")
    g1st_d = nc.dram_tensor("g1st", [3, 64], f32, kind="ExternalInput")
    g1b_d = nc.dram_tensor("g1b", [1, 64], f32, kind="ExternalInput")
    g2w_d = nc.dram_tensor("g2w", [64, 32], f32, kind="ExternalInput")
    g2b_d = nc.dram_tensor("g2b", [1, 32], f32, kind="ExternalInput")
    g3w_d = nc.dram_tensor("g3w", [32, 1], f32, kind="ExternalInput")
    g3b_d = nc.dram_tensor("g3b", [1, 1], f32, kind="ExternalInput")
    y_d = nc.dram_tensor("y", [C, HW_N], f32, kind="ExternalOutput")

    with tile.TileContext(nc) as tc:
        with (
            tc.tile_pool(name="persist", bufs=1) as persist,
            tc.tile_pool(name="qkbuf", bufs=1) as qkbuf,
            tc.tile_pool(name="expp", bufs=6) as expp,
            tc.tile_pool(name="small", bufs=8) as small,
            tc.tile_pool(name="rdp", bufs=4) as rdp,
            tc.tile_pool(name="rdbp", bufs=4) as rdbp,
            tc.tile_pool(name="ps_s", bufs=3, space="PSUM") as ps_s,
            tc.tile_pool(name="ps_pv", bufs=2, space="PSUM") as ps_pv,
        ):
            # ---- load inputs ------------------------------------------------
            xvv = persist.tile([C, HW_N], f32)
            xvh = persist.tile([C, HW_N], f32)
            coh = persist.tile([1, HW_N], f32)
            wq = persist.tile([C, NPAIR, C], f32)
            wk = persist.tile([C, NPAIR, C], f32)
            bq = persist.tile([C, NPAIR], f32)
            bk = persist.tile([C, NPAIR], f32)
            wv = persist.tile([C, C], f32)
            bvb = persist.tile([C, C], f32)
            woa = persist.tile([C, C], f32)
            wob = persist.tile([C, C], f32)
            bo = persist.tile([C, 1], f32)
            g1vv = persist.tile([C, 64], f32)
            g1vh = persist.tile([C, 64], f32)
            g1st = persist.tile([3, 64], f32)
            g1b = persist.tile([1, 64], f32)
            g2w = persist.tile([64, 32], f32)
            g2b = persist.tile([1, 32], f32)
            g3w = persist.tile([32, 1], f32)
            g3b = persist.tile([1, 1], f32)

            nc.sync.dma_start(out=xvv, in_=x_vv_d[:, :])
            nc.sync.dma_start(out=xvh, in_=x_vh_d[:, :])
            nc.sync.dma_start(out=coh, in_=coh_d[:, :])
            nc.sync.dma_start(out=wq, in_=wq_d.ap().rearrange("p (t m) -> p t m", t=NPAIR))
            nc.sync.dma_start(out=wk, in_=wk_d.ap().rearrange("p (t m) -> p t m", t=NPAIR))
            nc.sync.dma_start(out=bq, in_=bq_d[:, :])
            nc.sync.dma_start(out=bk, in_=bk_d[:, :])
            nc.sync.dma_start(out=wv, in_=wv_d[:, :])
            nc.sync.dma_start(out=bvb, in_=bv_d[:, :])
            nc.sync.dma_start(out=woa, in_=woa_d[:, :])
            nc.sync.dma_start(out=wob, in_=wob_d[:, :])
            nc.sync.dma_start(out=bo, in_=bo_d[:, :])
            nc.sync.dma_start(out=g1vv, in_=g1vv_d[:, :])
            nc.sync.dma_start(out=g1vh, in_=g1vh_d[:, :])
            nc.sync.dma_start(out=g1st, in_=g1st_d[:, :])
            nc.sync.dma_start(out=g1b, in_=g1b_d[:, :])
            nc.sync.dma_start(out=g2w, in_=g2w_d[:, :])
            nc.sync.dma_start(out=g2b, in_=g2b_d[:, :])
            nc.sync.dma_start(out=g3w, in_=g3w_d[:, :])
            nc.sync.dma_start(out=g3b, in_=g3b_d[:, :])

            ones = persist.tile([1, C], f32)
            nc.vector.memset(ones, 1.0)
            one1 = ones[:, 0:1]
            ones_col = persist.tile([C, 1], f32)
            nc.vector.memset(ones_col, 1.0)
            zero_col = persist.tile([C, 1], f32)
            nc.vector.memset(zero_col, 0.0)

            # rounded f32r views for the f32r projection matmuls
            xvv_r = persist.tile([C, HW_N], f32r)
            nc.vector.tensor_copy(out=xvv_r, in_=xvv)
            xvh_r = persist.tile([C, HW_N], f32r)
            nc.vector.tensor_copy(out=xvh_r, in_=xvh)
            wq_r = persist.tile([C, NPAIR, C], f32r)
            nc.vector.tensor_copy(out=wq_r, in_=wq)
            wk_r = persist.tile([C, NPAIR, C], f32r)
            nc.vector.tensor_copy(out=wk_r, in_=wk)
            wv_r = persist.tile([C, C], f32r)
            nc.vector.tensor_copy(out=wv_r, in_=wv)

            # ---- coherence weights w + stats -------------------------------
            cmx = small.tile([1, 1], f32)
            nc.vector.tensor_reduce(out=cmx, in_=coh, axis=mybir.AxisListType.X, op=OP.max)
            cmn = small.tile([1, 1], f32)
            nc.vector.tensor_reduce(out=cmn, in_=coh, axis=mybir.AxisListType.X, op=OP.min)
            rng = small.tile([1, 1], f32)
            # (cmax + 1e-8) - cmin
            nc.vector.scalar_tensor_tensor(
                out=rng, in0=cmx, scalar=1e-8, in1=cmn, op0=OP.add, op1=OP.subtract
            )
            rcp = small.tile([1, 1], f32)
            nc.vector.reciprocal(out=rcp, in_=rng)
            w_sb = persist.tile([1, HW_N], f32)
            nc.vector.tensor_scalar(
                out=w_sb, in0=coh, scalar1=cmn, scalar2=rcp, op0=OP.subtract, op1=OP.mult
            )

            # stats: s1 = sum(w), s2 = sum(w^2), wmx = max(w)
            s1 = small.tile([1, 1], f32)
            nc.vector.tensor_reduce(out=s1, in_=w_sb, axis=mybir.AxisListType.X, op=OP.add)
            w2 = small.tile([1, HW_N], f32, tag="wrow")
            nc.vector.tensor_mul(out=w2, in0=w_sb, in1=w_sb)
            s2 = small.tile([1, 1], f32)
            nc.vector.tensor_reduce(out=s2, in_=w2, axis=mybir.AxisListType.X, op=OP.add)
            wmx = small.tile([1, 1], f32)
            nc.vector.tensor_reduce(out=wmx, in_=w_sb, axis=mybir.AxisListType.X, op=OP.max)
            # std = sqrt(s2/N - (s1/N)^2) via exp(0.5*ln(var))
            m1 = small.tile([1, 1], f32)
            nc.vector.tensor_scalar_mul(out=m1, in0=s1, scalar1=1.0 / HW_N)
            msq = small.tile([1, 1], f32)
            nc.vector.tensor_mul(out=msq, in0=m1, in1=m1)
            var = small.tile([1, 1], f32)
            nc.vector.scalar_tensor_tensor(
                out=var, in0=s2, scalar=1.0 / HW_N, in1=msq, op0=OP.mult, op1=OP.subtract
            )
            lnv = small.tile([1, 1], f32)
            nc.scalar.activation(out=lnv, in_=var, func=AF.Ln)
            std = small.tile([1, 1], f32)
            nc.scalar.activation(out=std, in_=lnv, func=AF.Exp, scale=0.5)
            # stats row [1,3] = [s1, std, max]  (g1st row 0 is pre-scaled by 1/N)
            strow = small.tile([1, 3], f32)
            nc.vector.tensor_copy(out=strow[:, 0:1], in_=s1)
            nc.vector.tensor_copy(out=strow[:, 1:2], in_=std)
            nc.vector.tensor_copy(out=strow[:, 2:3], in_=wmx)
            ps_st = ps_pv.tile([C, NCHUNK], f32, tag="pv")
            nc.tensor.matmul(out=ps_st[0:3, 0:1], lhsT=strow, rhs=one1,
                             start=True, stop=True, tile_position=(0, 0))
            stcol = small.tile([3, 1], f32)
            nc.vector.tensor_copy(out=stcol, in_=ps_st[0:3, 0:1])

            # ---- gate MLP ---------------------------------------------------
            vvs = small.tile([C, 1], f32, tag="col")
            nc.vector.tensor_reduce(out=vvs, in_=xvv, axis=mybir.AxisListType.X, op=OP.add)
            vhs = small.tile([C, 1], f32, tag="col")
            nc.vector.tensor_reduce(out=vhs, in_=xvh, axis=mybir.AxisListType.X, op=OP.add)

            ps_g = ps_pv.tile([C, NCHUNK], f32, tag="pv")
            h1p = ps_g[0:64, 0:1]
            nc.tensor.matmul(out=h1p, lhsT=g1vv, rhs=vvs, start=True, stop=False,
                             tile_position=(0, 0))
            nc.tensor.matmul(out=h1p, lhsT=g1vh, rhs=vhs, start=False, stop=False,
                             tile_position=(0, 0))
            nc.tensor.matmul(out=h1p, lhsT=g1st, rhs=stcol, start=False, stop=False,
                             tile_position=(0, 0))
            nc.tensor.matmul(out=h1p, lhsT=g1b, rhs=one1, start=False, stop=True,
                             tile_position=(0, 0))
            h1 = small.tile([64, 1], f32, tag="col")
            nc.scalar.activation(out=h1, in_=h1p, func=AF.Relu)

            ps_g2 = ps_pv.tile([C, NCHUNK], f32, tag="pv")
            h2p = ps_g2[0:32, 0:1]
            nc.tensor.matmul(out=h2p, lhsT=g2w, rhs=h1, start=True, stop=False,
                             tile_position=(0, 0))
            nc.tensor.matmul(out=h2p, lhsT=g2b, rhs=one1, start=False, stop=True,
                             tile_position=(0, 0))
            h2 = small.tile([32, 1], f32, tag="col")
            nc.scalar.activation(out=h2, in_=h2p, func=AF.Relu)

            ps_g3 = ps_pv.tile([C, NCHUNK], f32, tag="pv")
            zp = ps_g3[0:1, 0:1]
            nc.tensor.matmul(out=zp, lhsT=g3w, rhs=h2, start=True, stop=False,
                             tile_position=(0, 0))
            nc.tensor.matmul(out=zp, lhsT=g3b, rhs=one1, start=False, stop=True,
                             tile_position=(0, 0))
            # g = sigmoid(z) = 1/(1+exp(-z))
            ez = small.tile([1, 1], f32)
            nc.scalar.activation(out=ez, in_=zp, func=AF.Exp, scale=-1.0)
            gden = small.tile([1, 1], f32)
            nc.vector.tensor_scalar_add(out=gden, in0=ez, scalar1=1.0)
            gsc = small.tile([1, 1], f32)
            nc.vector.reciprocal(out=gsc, in_=gden)
            # broadcast g to all partitions
            ps_gb = ps_pv.tile([C, NCHUNK], f32, tag="pv")
            nc.tensor.matmul(out=ps_gb[:, 0:1], lhsT=ones, rhs=gsc,
                             start=True, stop=True, tile_position=(0, 0))
            g_col = persist.tile([C, 1], f32)
            nc.vector.tensor_copy(out=g_col, in_=ps_gb[:, 0:1])
            # c = (1-g)/g = 1/g - 1  (scalar, partition 0); wc_row = c*w
            rg1 = small.tile([1, 1], f32)
            nc.vector.reciprocal(out=rg1, in_=gsc)
            c1 = small.tile([1, 1], f32)
            nc.vector.tensor_scalar_add(out=c1, in0=rg1, scalar1=-1.0)
            wc_row = persist.tile([1, HW_N], bf16)
            nc.vector.tensor_scalar_mul(out=wc_row, in0=w_sb, scalar1=c1)
            w_r = persist.tile([1, HW_N], bf16)
            nc.vector.tensor_copy(out=w_r, in_=w_sb)

            # ---- V' projection (token-major, 17-col head groups + ones) ----
            # vp[p, mc, h, 0] = 1;  vp[p, mc, h, 1:17] = V_seq[mc*128+p, 16h:16h+16]
            # (ones first so the PV denominator row lands on the 32-aligned
            #  strip base - engine APs must start at partition 0/32/64/96)
            vp = persist.tile([C, MT, NH, HD + 1], bf16)
            # memset can't write f32r; broadcast-copy rounds f32 -> f32r
            nc.vector.tensor_copy(out=vp[:, :, :, 0:1],
                                  in_=ones_col.to_broadcast([C, MT, NH, 1]))
            for gp in range(2):
                ps_v = ps_pv.tile([C, NCHUNK], f32, tag="pv")
                for i in range(4):
                    mc = 4 * gp + i
                    nc.tensor.matmul(
                        out=ps_v[:, i * C : (i + 1) * C],
                        lhsT=xvh_r[:, mc * C : (mc + 1) * C],
                        rhs=wv_r,
                        start=True, stop=True, tile_position=(0, 0),
                    )
                for i in range(4):
                    mc = 4 * gp + i
                    nc.vector.tensor_add(
                        out=vp[:, mc, :, 1 : HD + 1],
                        in0=ps_v[:, i * C : (i + 1) * C].rearrange(
                            "p (h d) -> p h d", h=NH
                        ),
                        in1=bvb.rearrange("p (h d) -> p h d", h=NH),
                    )

            # ---- Q^T / K^T projections (pair-padded head layout) -----------
            qt = qkbuf.tile([C, NPAIR, HW_N], bf16)
            kt = qkbuf.tile([C, NPAIR, HW_N], bf16)
            for t in range(NPAIR):
                ps_q = ps_s.tile([C, HW_N], f32, tag="s")
                for ncb in range(2):
                    sl = slice(ncb * NCHUNK, (ncb + 1) * NCHUNK)
                    nc.tensor.matmul(out=ps_q[:, sl], lhsT=wq_r[:, t, :],
                                     rhs=xvv_r[:, sl],
                                     start=True, stop=True, tile_position=(0, 0))
                nc.vector.tensor_scalar_add(out=qt[:, t, :], in0=ps_q, scalar1=bq[:, t : t + 1])
            for t in range(NPAIR):
                ps_k = ps_s.tile([C, HW_N], f32, tag="s")
                for ncb in range(2):
                    sl = slice(ncb * NCHUNK, (ncb + 1) * NCHUNK)
                    nc.tensor.matmul(out=ps_k[:, sl], lhsT=wk_r[:, t, :],
                                     rhs=xvh_r[:, sl],
                                     start=True, stop=True, tile_position=(0, 0))
                nc.vector.tensor_scalar_add(out=kt[:, t, :], in0=ps_k, scalar1=bk[:, t : t + 1])

            # The blended scores are computed as one K=17 contraction:
            # row 16 of each strip of kt holds w, of qt holds c*w, so
            # S^T + c*w w^T comes out of a single matmul. Engine writes
            # can't target partition 16/80 (alignment), but DMA can.
            for t in range(NPAIR):
                for s_ in (HD, 64 + HD):
                    nc.sync.dma_start(out=kt[s_ : s_ + 1, t, :], in_=w_r)
                    nc.sync.dma_start(out=qt[s_ : s_ + 1, t, :], in_=wc_row)

            # ---- attention: S^T -> exp -> PV, flash-style ------------------
            aoutA = persist.tile([C, HW_N], bf16)
            aoutB = persist.tile([C, HW_N], bf16)
            nc.vector.tensor_copy(out=aoutA, in_=zero_col.to_broadcast([C, HW_N]))
            nc.vector.tensor_copy(out=aoutB, in_=zero_col.to_broadcast([C, HW_N]))
            for t in range(NPAIR):
                # two heads per PV psum tile at col-strip bases 0 and 64 so the
                # pair's PV matmuls run concurrently in distinct col groups.
                pv_tiles = {
                    ncb: ps_pv.tile(
                        [C, NCHUNK], f32, tag="pv", name=f"pv_{t}_{ncb}"
                    )
                    for ncb in range(2)
                }
                for mt_i in range(MT):
                    msl = slice(mt_i * C, (mt_i + 1) * C)
                    ps_pair = []
                    for si, s in enumerate((0, 64)):
                        ps_pair.append(ps_s.tile([C, HW_N], f32, tag="s",
                                                 name=f"s_{t}_{mt_i}_{si}"))
                    # alternate strips so row-group concurrency kicks in
                    for ncb in range(2):
                        sl = slice(ncb * NCHUNK, (ncb + 1) * NCHUNK)
                        for si, s in enumerate((0, 64)):
                            nc.tensor.matmul(
                                out=ps_pair[si][:, sl],
                                lhsT=kt[s : s + HD + 1, t, msl],
                                rhs=qt[s : s + HD + 1, t, sl],
                                start=True, stop=True, tile_position=(s, 0),
                            )
                    es_tiles = []
                    for si in range(2):
                        es = expp.tile([C, HW_N], bf16, tag="es", name=f"es_{si}")
                        nc.scalar.activation(out=es, in_=ps_pair[si], func=AF.Exp,
                                             scale=g_col)
                        es_tiles.append(es)
                    # alternate col strips for PV concurrency; disjoint-partition
                    # psum groups are HW-safe (per-element has_written).
                    for ncb in range(2):
                        sl = slice(ncb * NCHUNK, (ncb + 1) * NCHUNK)
                        for si in range(2):
                            nc.tensor.matmul(
                                out=pv_tiles[ncb][64 * si : 64 * si + HD + 1, :],
                                lhsT=vp[:, mt_i, 2 * t + si, :],
                                rhs=es_tiles[si][:, sl],
                                start=(mt_i == 0), stop=(mt_i == MT - 1),
                                tile_position=(0, 64 * si),
                                skip_group_check=True,
                            )
                # normalize this pair's two heads
                for si in range(2):
                    h = 2 * t + si
                    j = h % 4
                    for ncb in range(2):
                        pvt = pv_tiles[ncb]
                        rd = rdp.tile([1, NCHUNK], f32, tag="rd")
                        nc.vector.reciprocal_approx_fast(
                            out=rd, in_=pvt[64 * si : 64 * si + 1, :]
                        )
                        rdb = rdbp.tile([HD + 1, NCHUNK], f32, tag="rdb")
                        nc.gpsimd.partition_broadcast(rdb, rd)
                        # denom*recip(denom) junk row hits a zero Wo row.
                        dst = aoutA if h < 4 else aoutB
                        nc.vector.tensor_mul(
                            out=dst[32 * j : 32 * j + HD + 1,
                                    ncb * NCHUNK : (ncb + 1) * NCHUNK],
                            in0=pvt[64 * si : 64 * si + HD + 1, :],
                            in1=rdb,
                        )

            # rounded copies of the padded output-projection weights
            woa_r = persist.tile([C, C], bf16)
            nc.vector.tensor_copy(out=woa_r, in_=woa)
            wob_r = persist.tile([C, C], bf16)
            nc.vector.tensor_copy(out=wob_r, in_=wob)

            # ---- output projection + bias + residual -----------------------
            ps_y = ps_s.tile([C, HW_N], f32, tag="s")
            for ncb in range(2):
                sl = slice(ncb * NCHUNK, (ncb + 1) * NCHUNK)
                nc.tensor.matmul(out=ps_y[:, sl], lhsT=woa_r, rhs=aoutA[:, sl],
                                 start=True, stop=False, tile_position=(0, 0))
                nc.tensor.matmul(out=ps_y[:, sl], lhsT=wob_r, rhs=aoutB[:, sl],
                                 start=False, stop=True, tile_position=(0, 0))
            y_sb = persist.tile([C, HW_N], f32)
            # (ps_y + bo) + x_vv
            nc.vector.scalar_tensor_tensor(
                out=y_sb, in0=ps_y, scalar=bo, in1=xvv, op0=OP.add, op1=OP.add
            )
            nc.sync.dma_start(out=y_d[:, :], in_=y_sb)

    nc.compile()
    _CACHE["nc"] = nc
    return nc


def make_in_maps(inputs: dict) -> list[dict]:
    """Host-side prep: shard over batch, pre-transpose/pad the small weights."""
    f32 = np.float32
    vv = np.ascontiguousarray(inputs["vv_features"], dtype=f32)
    vh = np.ascontiguousarray(inputs["vh_features"], dtype=f32)
    coh = np.ascontiguousarray(inputs["coherence_matrix"], dtype=f32)
    Wq = np.asarray(inputs["Wq"], f32)
    bq = np.asarray(inputs["bq"], f32)
    Wk = np.asarray(inputs["Wk"], f32)
    bk = np.asarray(inputs["bk"], f32)
    Wv = np.asarray(inputs["Wv"], f32)
    bv = np.asarray(inputs["bv"], f32)
    Wo = np.asarray(inputs["Wo"], f32)
    bo = np.asarray(inputs["bo"], f32)
    temp = float(np.asarray(inputs["temperature"], f32).reshape(-1)[0])
    g1w = np.asarray(inputs["g1w"], f32)
    g1b = np.asarray(inputs["g1b"], f32)
    g2w = np.asarray(inputs["g2w"], f32)
    g2b = np.asarray(inputs["g2b"], f32)
    g3w = np.asarray(inputs["g3w"], f32)
    g3b = np.asarray(inputs["g3b"], f32)

    def _wo_pad(Wo_, grp):
        # lhsT [c_in_padded=128, c_out=128]: strip j row d holds Wo column for
        # channel 16*(4*grp+j)+d; pad rows (d>=16) are zero.
        wp = np.zeros((C, C), f32)
        for j in range(4):
            ch0 = HD * (4 * grp + j)
            wp[32 * j + 1 : 32 * j + 1 + HD, :] = Wo_[:, ch0 : ch0 + HD].T
        return np.ascontiguousarray(wp)

    def pad_pair(Wt, bt):
        # -> lhsT tiles [C, NPAIR, C] flattened to [C, NPAIR*C]; bias [C, NPAIR]
        wpad = np.zeros((NPAIR, C, C), f32)   # [t, c_in, m]
        bpad = np.zeros((C, NPAIR), f32)
        for t in range(NPAIR):
            wpad[t, :, 0:HD] = Wt[2 * t * HD : (2 * t + 1) * HD, :].T
            wpad[t, :, 64 : 64 + HD] = Wt[(2 * t + 1) * HD : (2 * t + 2) * HD, :].T
            bpad[0:HD, t] = bt[2 * t * HD : (2 * t + 1) * HD]
            bpad[64 : 64 + HD, t] = bt[(2 * t + 1) * HD : (2 * t + 2) * HD]
        wflat = np.ascontiguousarray(wpad.transpose(1, 0, 2).reshape(C, NPAIR * C))
        return wflat, np.ascontiguousarray(bpad)

    wq_h, bq_h = pad_pair(Wq * temp, bq * temp)
    wk_h, bk_h = pad_pair(Wk, bk)

    shared = {
        "wq": wq_h, "bq": bq_h, "wk": wk_h, "bk": bk_h,
        "wv": np.ascontiguousarray(Wv.T), "bv": np.ascontiguousarray(np.tile(bv, (C, 1))),
        "woa": _wo_pad(Wo, 0), "wob": _wo_pad(Wo, 1),
        "bo": np.ascontiguousarray(bo[:, None]),
        "g1vv": np.ascontiguousarray(g1w[:, :C].T / HW_N),
        "g1vh": np.ascontiguousarray(g1w[:, C : 2 * C].T / HW_N),
        "g1st": np.ascontiguousarray(g1w[:, 2 * C : 2 * C + 3].T * np.array([1.0 / HW_N, 1.0, 1.0], f32)[:, None]),
        "g1b": np.ascontiguousarray(g1b[None, :]),
        "g2w": np.ascontiguousarray(g2w.T), "g2b": np.ascontiguousarray(g2b[None, :]),
        "g3w": np.ascontiguousarray(g3w.T), "g3b": np.ascontiguousarray(g3b[None, :]),
    }
    in_maps = []
    for b in range(B):
        m = dict(shared)
        m["x_vv"] = np.ascontiguousarray(vv[b].reshape(C, HW_N))
        m["x_vh"] = np.ascontiguousarray(vh[b].reshape(C, HW_N))
        m["coh"] = np.ascontiguousarray(coh[b].reshape(1, HW_N))
        in_maps.append(m)
    return in_maps


def kernel(**inputs) -> np.ndarray:
    nc = build_program()
    in_maps = make_in_maps(inputs)
    from concourse.bass_utils import run_bass_kernel_spmd

    res = run_bass_kernel_spmd(nc, in_maps, core_ids=list(range(NCORES)))
    out = np.stack([res.results[i]["y"].reshape(C, 32, 32) for i in range(B)])
    return np.ascontiguousarray(out.astype(np.float32))


# revision 13
# speedup vs baseline: 3.7690x; 1.2035x over previous
"""CoherenceGuidedAttention Trainium2 Bass kernel.

Sharding: data-parallel over batch B=8 -> one batch per NeuronCore (8 cores).
All parameters replicated; no collectives.

Per-core algorithm (flash-style, channel-major "layout B"):
  X_vv, X_vh       : [C=128, N=1024] SBUF (natural layout of [C,H,W] input)
  Q^T = temp*(Wq X_vv + bq), K^T = Wk X_vh + bk   (pair-padded head layout)
  V'  = (X_vh^T Wv + bv | ones)  token-major with per-head 17-col groups
  S^T(h) = K_h Q_h^T   computed per (head, m-tile) into PSUM [128,1024]
           + rank-1 accumulate  c*w w^T  (c=(1-g)/g) via K=1 matmuls
  expS = exp(g * S^T_blend)  on ACT directly from PSUM (scale = per-partition g)
  PV   = V'_h^T expS  accumulated over m-chunks -> [17, 512] (row 16 = denom)
  Aout^T[h] = PV[0:16] * (1/denom)  (fast reciprocal + partition broadcast)
  Y^T = Wo Aout^T + bo + X_vv  -> output [C, N]

Head layout for PE row-tiling: Q^T/K^T stored as 4 pair-tiles [128, 1024];
pair t holds head 2t at partitions 0:16 and head 2t+1 at partitions 64:80,
so the two heads' QK^T matmuls run concurrently in PE row-strips 0 and 64.
"""

import sys

if "/opt/trn_rl_repo" not in sys.path:
    sys.path.insert(0, "/opt/trn_rl_repo")

import numpy as np

B = 8
C = 128
HW_N = 1024  # H*W tokens
NH = 8       # heads
HD = 16      # head dim
NCORES = 8
NPAIR = 4    # head pairs
MT = 8       # m tiles of 128 tokens
NCHUNK = 512  # fp32 moving-operand max

_CACHE: dict = {}


def build_program():
    """Build (once) the SPMD Bass program for one core."""
    if "nc" in _CACHE:
        return _CACHE["nc"]

    from concourse import bacc
    import concourse.mybir as mybir
    import concourse.tile as tile

    f32 = mybir.dt.float32
    f32r = mybir.dt.float32r
    bf16 = mybir.dt.bfloat16
    AF = mybir.ActivationFunctionType
    OP = mybir.AluOpType

    nc = bacc.Bacc("TRN2", target_bir_lowering=False, debug=False)

    # ---- DRAM I/O ----------------------------------------------------------
    x_vv_d = nc.dram_tensor("x_vv", [C, HW_N], f32, kind="ExternalInput")
    x_vh_d = nc.dram_tensor("x_vh", [C, HW_N], f32, kind="ExternalInput")
    coh_d = nc.dram_tensor("coh", [1, HW_N], f32, kind="ExternalInput")
    wq_d = nc.dram_tensor("wq", [C, 2 * C], f32, kind="ExternalInput")
    wk_d = nc.dram_tensor("wk", [C, 2 * C], f32, kind="ExternalInput")
    bq_d = nc.dram_tensor("bq", [C, 2], f32, kind="ExternalInput")
    bk_d = nc.dram_tensor("bk", [C, 2], f32, kind="ExternalInput")
    wv_d = nc.dram_tensor("wv", [C, C], f32, kind="ExternalInput")
    bv_d = nc.dram_tensor("bv", [C, C], f32, kind="ExternalInput")
    woa_d = nc.dram_tensor("woa", [C, C], f32, kind="ExternalInput")
    wob_d = nc.dram_tensor("wob", [C, C], f32, kind="ExternalInput")
    bo_d = nc.dram_tensor("bo", [C, 1], f32, kind="ExternalInput")
    g1vv_d = nc.dram_tensor("g1vv", [C, 64], f32, kind="ExternalInput")
    g1vh_d = nc.dram_tensor("g1vh", [C, 64], f32, kind="ExternalInput")
    g1st_d = nc.dram_tensor("g1st", [3, 64], f32, kind="ExternalInput")
    g1b_d = nc.dram_tensor("g1b", [1, 64], f32, kind="ExternalInput")
    g2w_d = nc.dram_tensor("g2w", [64, 32], f32, kind="ExternalInput")
    g2b_d = nc.dram_tensor("g2b", [1, 32], f32, kind="ExternalInput")
    g3w_d = nc.dram_tensor("g3w", [32, 1], f32, kind="ExternalInput")
    g3b_d = nc.dram_tensor("g3b", [1, 1], f32, kind="ExternalInput")
    y_d = nc.dram_tensor("y", [C, HW_N], f32, kind="ExternalOutput")

    with tile.TileContext(nc) as tc:
        with (
            tc.tile_pool(name="persist", bufs=1) as persist,
            tc.tile_pool(name="qkbuf", bufs=1) as qkbuf,
            tc.tile_pool(name="expp", bufs=6) as expp,
            tc.tile_pool(name="small", bufs=8) as small,
            tc.tile_pool(name="rdp", bufs=4) as rdp,
            tc.tile_pool(name="rdbp", bufs=4) as rdbp,
            tc.tile_pool(name="ps_s", bufs=3, space="PSUM") as ps_s,
            tc.tile_pool(name="ps_pv", bufs=2, space="PSUM") as ps_pv,
        ):
            # ---- load inputs ------------------------------------------------
            xvv = persist.tile([C, HW_N], f32)
            xvh = persist.tile([C, HW_N], f32)
            coh = persist.tile([1, HW_N], f32)
            wq = persist.tile([C, 2, C], f32)
            wk = persist.tile([C, 2, C], f32)
            bq = persist.tile([C, 2], f32)
            bk = persist.tile([C, 2], f32)
            wv = persist.tile([C, C], f32)
            bvb = persist.tile([C, C], f32)
            woa = persist.tile([C, C], f32)
            wob = persist.tile([C, C], f32)
            bo = persist.tile([C, 1], f32)
            g1vv = persist.tile([C, 64], f32)
            g1vh = persist.tile([C, 64], f32)
            g1st = persist.tile([3, 64], f32)
            g1b = persist.tile([1, 64], f32)
            g2w = persist.tile([64, 32], f32)
            g2b = persist.tile([1, 32], f32)
            g3w = persist.tile([32, 1], f32)
            g3b = persist.tile([1, 1], f32)

            nc.sync.dma_start(out=xvv, in_=x_vv_d[:, :])
            nc.sync.dma_start(out=xvh, in_=x_vh_d[:, :])
            nc.sync.dma_start(out=coh, in_=coh_d[:, :])
            nc.sync.dma_start(out=wq, in_=wq_d.ap().rearrange("p (t m) -> p t m", t=2))
            nc.sync.dma_start(out=wk, in_=wk_d.ap().rearrange("p (t m) -> p t m", t=2))
            nc.sync.dma_start(out=bq, in_=bq_d[:, :])
            nc.sync.dma_start(out=bk, in_=bk_d[:, :])
            nc.sync.dma_start(out=wv, in_=wv_d[:, :])
            nc.sync.dma_start(out=bvb, in_=bv_d[:, :])
            nc.sync.dma_start(out=woa, in_=woa_d[:, :])
            nc.sync.dma_start(out=wob, in_=wob_d[:, :])
            nc.sync.dma_start(out=bo, in_=bo_d[:, :])
            nc.sync.dma_start(out=g1vv, in_=g1vv_d[:, :])
            nc.sync.dma_start(out=g1vh, in_=g1vh_d[:, :])
            nc.sync.dma_start(out=g1st, in_=g1st_d[:, :])
            nc.sync.dma_start(out=g1b, in_=g1b_d[:, :])
            nc.sync.dma_start(out=g2w, in_=g2w_d[:, :])
            nc.sync.dma_start(out=g2b, in_=g2b_d[:, :])
            nc.sync.dma_start(out=g3w, in_=g3w_d[:, :])
            nc.sync.dma_start(out=g3b, in_=g3b_d[:, :])

            ones = persist.tile([1, C], f32)
            nc.vector.memset(ones, 1.0)
            one1 = ones[:, 0:1]
            ones_col = persist.tile([C, 1], f32)
            nc.vector.memset(ones_col, 1.0)
            zero_col = persist.tile([C, 1], f32)
            nc.vector.memset(zero_col, 0.0)

            # rounded f32r views for the f32r projection matmuls
            xvv_r = persist.tile([C, HW_N], f32r)
            nc.vector.tensor_copy(out=xvv_r, in_=xvv)
            xvh_r = persist.tile([C, HW_N], f32r)
            nc.vector.tensor_copy(out=xvh_r, in_=xvh)
            wq_r = persist.tile([C, 2, C], f32r)
            nc.vector.tensor_copy(out=wq_r, in_=wq)
            wk_r = persist.tile([C, 2, C], f32r)
            nc.vector.tensor_copy(out=wk_r, in_=wk)
            wv_r = persist.tile([C, C], f32r)
            nc.vector.tensor_copy(out=wv_r, in_=wv)

            # ---- coherence weights w + stats -------------------------------
            cmx = small.tile([1, 1], f32)
            nc.vector.tensor_reduce(out=cmx, in_=coh, axis=mybir.AxisListType.X, op=OP.max)
            cmn = small.tile([1, 1], f32)
            nc.vector.tensor_reduce(out=cmn, in_=coh, axis=mybir.AxisListType.X, op=OP.min)
            rng = small.tile([1, 1], f32)
            # (cmax + 1e-8) - cmin
            nc.vector.scalar_tensor_tensor(
                out=rng, in0=cmx, scalar=1e-8, in1=cmn, op0=OP.add, op1=OP.subtract
            )
            rcp = small.tile([1, 1], f32)
            nc.vector.reciprocal(out=rcp, in_=rng)
            w_sb = persist.tile([1, HW_N], f32)
            nc.vector.tensor_scalar(
                out=w_sb, in0=coh, scalar1=cmn, scalar2=rcp, op0=OP.subtract, op1=OP.mult
            )

            # stats: s1 = sum(w), s2 = sum(w^2), wmx = max(w)
            s1 = small.tile([1, 1], f32)
            nc.vector.tensor_reduce(out=s1, in_=w_sb, axis=mybir.AxisListType.X, op=OP.add)
            w2 = small.tile([1, HW_N], f32, tag="wrow")
            nc.vector.tensor_mul(out=w2, in0=w_sb, in1=w_sb)
            s2 = small.tile([1, 1], f32)
            nc.vector.tensor_reduce(out=s2, in_=w2, axis=mybir.AxisListType.X, op=OP.add)
            wmx = small.tile([1, 1], f32)
            nc.vector.tensor_reduce(out=wmx, in_=w_sb, axis=mybir.AxisListType.X, op=OP.max)
            # std = sqrt(s2/N - (s1/N)^2) via exp(0.5*ln(var))
            m1 = small.tile([1, 1], f32)
            nc.vector.tensor_scalar_mul(out=m1, in0=s1, scalar1=1.0 / HW_N)
            msq = small.tile([1, 1], f32)
            nc.vector.tensor_mul(out=msq, in0=m1, in1=m1)
            var = small.tile([1, 1], f32)
            nc.vector.scalar_tensor_tensor(
                out=var, in0=s2, scalar=1.0 / HW_N, in1=msq, op0=OP.mult, op1=OP.subtract
            )
            lnv = small.tile([1, 1], f32)
            nc.scalar.activation(out=lnv, in_=var, func=AF.Ln)
            std = small.tile([1, 1], f32)
            nc.scalar.activation(out=std, in_=lnv, func=AF.Exp, scale=0.5)
            # stats row [1,3] = [s1, std, max]  (g1st row 0 is pre-scaled by 1/N)
            strow = small.tile([1, 3], f32)
            nc.vector.tensor_copy(out=strow[:, 0:1], in_=s1)
            nc.vector.tensor_copy(out=strow[:, 1:2], in_=std)
            nc.vector.tensor_copy(out=strow[:, 2:3], in_=wmx)
            ps_st = ps_pv.tile([C, NCHUNK], f32, tag="pv")
            nc.tensor.matmul(out=ps_st[0:3, 0:1], lhsT=strow, rhs=one1,
                             start=True, stop=True, tile_position=(0, 0))
            stcol = small.tile([3, 1], f32)
            nc.vector.tensor_copy(out=stcol, in_=ps_st[0:3, 0:1])

            # ---- gate MLP ---------------------------------------------------
            vvs = small.tile([C, 1], f32, tag="col")
            nc.vector.tensor_reduce(out=vvs, in_=xvv, axis=mybir.AxisListType.X, op=OP.add)
            vhs = small.tile([C, 1], f32, tag="col")
            nc.vector.tensor_reduce(out=vhs, in_=xvh, axis=mybir.AxisListType.X, op=OP.add)

            ps_g = ps_pv.tile([C, NCHUNK], f32, tag="pv")
            h1p = ps_g[0:64, 0:1]
            nc.tensor.matmul(out=h1p, lhsT=g1vv, rhs=vvs, start=True, stop=False,
                             tile_position=(0, 0))
            nc.tensor.matmul(out=h1p, lhsT=g1vh, rhs=vhs, start=False, stop=False,
                             tile_position=(0, 0))
            nc.tensor.matmul(out=h1p, lhsT=g1st, rhs=stcol, start=False, stop=False,
                             tile_position=(0, 0))
            nc.tensor.matmul(out=h1p, lhsT=g1b, rhs=one1, start=False, stop=True,
                             tile_position=(0, 0))
            h1 = small.tile([64, 1], f32, tag="col")
            nc.scalar.activation(out=h1, in_=h1p, func=AF.Relu)

            ps_g2 = ps_pv.tile([C, NCHUNK], f32, tag="pv")
            h2p = ps_g2[0:32, 0:1]
            nc.tensor.matmul(out=h2p, lhsT=g2w, rhs=h1, start=True, stop=False,
                             tile_position=(0, 0))
            nc.tensor.matmul(out=h2p, lhsT=g2b, rhs=one1, start=False, stop=True,
                             tile_position=(0, 0))
            h2 = small.tile([32, 1], f32, tag="col")
            nc.scalar.activation(out=h2, in_=h2p, func=AF.Relu)

            ps_g3 = ps_pv.tile([C, NCHUNK], f32, tag="pv")
            zp = ps_g3[0:1, 0:1]
            nc.tensor.matmul(out=zp, lhsT=g3w, rhs=h2, start=True, stop=False,
                             tile_position=(0, 0))
            nc.tensor.matmul(out=zp, lhsT=g3b, rhs=one1, start=False, stop=True,
                             tile_position=(0, 0))
            # g = sigmoid(z) = 1/(1+exp(-z))
            ez = small.tile([1, 1], f32)
            nc.scalar.activation(out=ez, in_=zp, func=AF.Exp, scale=-1.0)
            gden = small.tile([1, 1], f32)
            nc.vector.tensor_scalar_add(out=gden, in0=ez, scalar1=1.0)
            gsc = small.tile([1, 1], f32)
            nc.vector.reciprocal(out=gsc, in_=gden)
            # broadcast g to all partitions
            ps_gb = ps_pv.tile([C, NCHUNK], f32, tag="pv")
            nc.tensor.matmul(out=ps_gb[:, 0:1], lhsT=ones, rhs=gsc,
                             start=True, stop=True, tile_position=(0, 0))
            g_col = persist.tile([C, 1], f32)
            nc.vector.tensor_copy(out=g_col, in_=ps_gb[:, 0:1])
            # c = (1-g)/g = 1/g - 1  (scalar, partition 0); wc_row = c*w
            rg1 = small.tile([1, 1], f32)
            nc.vector.reciprocal(out=rg1, in_=gsc)
            c1 = small.tile([1, 1], f32)
            nc.vector.tensor_scalar_add(out=c1, in0=rg1, scalar1=-1.0)
            wc_row = persist.tile([1, HW_N], bf16)
            nc.vector.tensor_scalar_mul(out=wc_row, in0=w_sb, scalar1=c1)
            w_r = persist.tile([1, HW_N], bf16)
            nc.vector.tensor_copy(out=w_r, in_=w_sb)

            # ---- V' projection (token-major, 17-col head groups + ones) ----
            # vp[p, mc, h, 0] = 1;  vp[p, mc, h, 1:17] = V_seq[mc*128+p, 16h:16h+16]
            # (ones first so the PV denominator row lands on the 32-aligned
            #  strip base - engine APs must start at partition 0/32/64/96)
            vp = persist.tile([C, MT, NH, HD + 1], bf16)
            # memset can't write f32r; broadcast-copy rounds f32 -> f32r
            nc.vector.tensor_copy(out=vp[:, :, :, 0:1],
                                  in_=ones_col.to_broadcast([C, MT, NH, 1]))
            for gp in range(2):
                ps_v = ps_pv.tile([C, NCHUNK], f32, tag="pv")
                for i in range(4):
                    mc = 4 * gp + i
                    nc.tensor.matmul(
                        out=ps_v[:, i * C : (i + 1) * C],
                        lhsT=xvh_r[:, mc * C : (mc + 1) * C],
                        rhs=wv_r,
                        start=True, stop=True, tile_position=(0, 0),
                    )
                for i in range(4):
                    mc = 4 * gp + i
                    nc.vector.tensor_add(
                        out=vp[:, mc, :, 1 : HD + 1],
                        in0=ps_v[:, i * C : (i + 1) * C].rearrange(
                            "p (h d) -> p h d", h=NH
                        ),
                        in1=bvb.rearrange("p (h d) -> p h d", h=NH),
                    )

            # ---- Q^T / K^T projections (pair-padded head layout) -----------
            qt = qkbuf.tile([C, 2, HW_N], bf16)
            kt = qkbuf.tile([C, 2, HW_N], bf16)
            for g in range(2):
                ps_q = ps_s.tile([C, HW_N], f32, tag="s")
                for ncb in range(2):
                    sl = slice(ncb * NCHUNK, (ncb + 1) * NCHUNK)
                    nc.tensor.matmul(out=ps_q[:, sl], lhsT=wq_r[:, g, :],
                                     rhs=xvv_r[:, sl],
                                     start=True, stop=True, tile_position=(0, 0))
                nc.vector.tensor_scalar_add(out=qt[:, g, :], in0=ps_q, scalar1=bq[:, g : g + 1])
            for g in range(2):
                ps_k = ps_s.tile([C, HW_N], f32, tag="s")
                for ncb in range(2):
                    sl = slice(ncb * NCHUNK, (ncb + 1) * NCHUNK)
                    nc.tensor.matmul(out=ps_k[:, sl], lhsT=wk_r[:, g, :],
                                     rhs=xvh_r[:, sl],
                                     start=True, stop=True, tile_position=(0, 0))
                nc.vector.tensor_scalar_add(out=kt[:, g, :], in0=ps_k, scalar1=bk[:, g : g + 1])

            # The blended scores are computed as one K=17 contraction:
            # row 16 of each strip of kt holds w, of qt holds c*w, so
            # S^T + c*w w^T comes out of a single matmul. Engine writes
            # can't target partition 16/80 (alignment), but DMA can.
            for g in range(2):
                for j in range(4):
                    s_ = 32 * j + HD
                    nc.sync.dma_start(out=kt[s_ : s_ + 1, g, :], in_=w_r)
                    nc.sync.dma_start(out=qt[s_ : s_ + 1, g, :], in_=wc_row)

            # ---- attention: S^T -> exp -> PV, flash-style ------------------
            aoutA = persist.tile([C, HW_N], bf16)
            aoutB = persist.tile([C, HW_N], bf16)
            nc.vector.tensor_copy(out=aoutA, in_=zero_col.to_broadcast([C, HW_N]))
            nc.vector.tensor_copy(out=aoutB, in_=zero_col.to_broadcast([C, HW_N]))
            for g in range(2):
                # one PV psum tile per n-chunk holds all 4 heads of the quad
                # at col strips 0/32/64/96 (4-way col-tiled PV matmuls).
                pv_tiles = {
                    ncb: ps_pv.tile(
                        [C, NCHUNK], f32, tag="pv", name=f"pv_{g}_{ncb}"
                    )
                    for ncb in range(2)
                }
                for mt_i in range(MT):
                    msl = slice(mt_i * C, (mt_i + 1) * C)
                    for ncb in range(2):
                        sl = slice(ncb * NCHUNK, (ncb + 1) * NCHUNK)
                        # two psum tiles per wave; each holds two heads'
                        # n-chunk so one exp unblocks two PV matmuls
                        pshalf = []
                        for half in range(2):
                            pshalf.append(ps_s.tile(
                                [C, HW_N], f32, tag="s",
                                name=f"s_{g}_{mt_i}_{ncb}_{half}"))
                        for j in range(4):
                            nc.tensor.matmul(
                                out=pshalf[j // 2][:, (j % 2) * NCHUNK
                                                   : (j % 2 + 1) * NCHUNK],
                                lhsT=kt[32 * j : 32 * j + HD + 1, g, msl],
                                rhs=qt[32 * j : 32 * j + HD + 1, g, sl],
                                start=True, stop=True, tile_position=(32 * j, 0),
                            )
                        es_tiles = []
                        for half in range(2):
                            es = expp.tile([C, HW_N], bf16, tag="es",
                                           name=f"es_{half}")
                            nc.scalar.activation(out=es, in_=pshalf[half],
                                                 func=AF.Exp, scale=g_col)
                            es_tiles.append(es)
                        for j in range(4):
                            nc.tensor.matmul(
                                out=pv_tiles[ncb][32 * j : 32 * j + HD + 1, :],
                                lhsT=vp[:, mt_i, 4 * g + j, :],
                                rhs=es_tiles[j // 2][:, (j % 2) * NCHUNK
                                                     : (j % 2 + 1) * NCHUNK],
                                start=(mt_i == 0), stop=(mt_i == MT - 1),
                                tile_position=(0, 32 * j),
                                skip_group_check=True,
                            )
                # normalize the quad's four heads
                for j in range(4):
                    h = 4 * g + j
                    for ncb in range(2):
                        pvt = pv_tiles[ncb]
                        rd = rdp.tile([1, NCHUNK], f32, tag="rd")
                        nc.vector.reciprocal_approx_fast(
                            out=rd, in_=pvt[32 * j : 32 * j + 1, :]
                        )
                        rdb = rdbp.tile([HD + 1, NCHUNK], f32, tag="rdb")
                        nc.gpsimd.partition_broadcast(rdb, rd)
                        # denom*recip(denom) junk row hits a zero Wo row.
                        dst = aoutA if g == 0 else aoutB
                        nc.vector.tensor_mul(
                            out=dst[32 * j : 32 * j + HD + 1,
                                    ncb * NCHUNK : (ncb + 1) * NCHUNK],
                            in0=pvt[32 * j : 32 * j + HD + 1, :],
                            in1=rdb,
                        )

            # rounded copies of the padded output-projection weights
            woa_r = persist.tile([C, C], bf16)
            nc.vector.tensor_copy(out=woa_r, in_=woa)
            wob_r = persist.tile([C, C], bf16)
            nc.vector.tensor_copy(out=wob_r, in_=wob)

            # ---- output projection + bias + residual -----------------------
            ps_y = ps_s.tile([C, HW_N], f32, tag="s")
            for ncb in range(2):
                sl = slice(ncb * NCHUNK, (ncb + 1) * NCHUNK)
                nc.tensor.matmul(out=ps_y[:, sl], lhsT=woa_r, rhs=aoutA[:, sl],
                                 start=True, stop=False, tile_position=(0, 0))
                nc.tensor.matmul(out=ps_y[:, sl], lhsT=wob_r, rhs=aoutB[:, sl],
                                 start=False, stop=True, tile_position=(0, 0))
            y_sb = persist.tile([C, HW_N], f32)
            # (ps_y + bo) + x_vv
            nc.vector.scalar_tensor_tensor(
                out=y_sb, in0=ps_y, scalar=bo, in1=xvv, op0=OP.add, op1=OP.add
            )
            nc.sync.dma_start(out=y_d[:, :], in_=y_sb)

    nc.compile()
    _CACHE["nc"] = nc
    return nc


def make_in_maps(inputs: dict) -> list[dict]:
    """Host-side prep: shard over batch, pre-transpose/pad the small weights."""
    f32 = np.float32
    vv = np.ascontiguousarray(inputs["vv_features"], dtype=f32)
    vh = np.ascontiguousarray(inputs["vh_features"], dtype=f32)
    coh = np.ascontiguousarray(inputs["coherence_matrix"], dtype=f32)
    Wq = np.asarray(inputs["Wq"], f32)
    bq = np.asarray(inputs["bq"], f32)
    Wk = np.asarray(inputs["Wk"], f32)
    bk = np.asarray(inputs["bk"], f32)
    Wv = np.asarray(inputs["Wv"], f32)
    bv = np.asarray(inputs["bv"], f32)
    Wo = np.asarray(inputs["Wo"], f32)
    bo = np.asarray(inputs["bo"], f32)
    temp = float(np.asarray(inputs["temperature"], f32).reshape(-1)[0])
    g1w = np.asarray(inputs["g1w"], f32)
    g1b = np.asarray(inputs["g1b"], f32)
    g2w = np.asarray(inputs["g2w"], f32)
    g2b = np.asarray(inputs["g2b"], f32)
    g3w = np.asarray(inputs["g3w"], f32)
    g3b = np.asarray(inputs["g3b"], f32)

    def _wo_pad(Wo_, grp):
        # lhsT [c_in_padded=128, c_out=128]: strip j row d holds Wo column for
        # channel 16*(4*grp+j)+d; pad rows (d>=16) are zero.
        wp = np.zeros((C, C), f32)
        for j in range(4):
            ch0 = HD * (4 * grp + j)
            wp[32 * j + 1 : 32 * j + 1 + HD, :] = Wo_[:, ch0 : ch0 + HD].T
        return np.ascontiguousarray(wp)

    def pad_quad(Wt, bt):
        # quad g holds head 4g+j at partitions 32j..32j+16 (w row at 32j+16)
        wpad = np.zeros((2, C, C), f32)   # [g, c_in, m]
        bpad = np.zeros((C, 2), f32)
        for g in range(2):
            for j in range(4):
                h = 4 * g + j
                wpad[g, :, 32 * j : 32 * j + HD] = Wt[h * HD : (h + 1) * HD, :].T
                bpad[32 * j : 32 * j + HD, g] = bt[h * HD : (h + 1) * HD]
        wflat = np.ascontiguousarray(wpad.transpose(1, 0, 2).reshape(C, 2 * C))
        return wflat, np.ascontiguousarray(bpad)

    wq_h, bq_h = pad_quad(Wq * temp, bq * temp)
    wk_h, bk_h = pad_quad(Wk, bk)

    shared = {
        "wq": wq_h, "bq": bq_h, "wk": wk_h, "bk": bk_h,
        "wv": np.ascontiguousarray(Wv.T), "bv": np.ascontiguousarray(np.tile(bv, (C, 1))),
        "woa": _wo_pad(Wo, 0), "wob": _wo_pad(Wo, 1),
        "bo": np.ascontiguousarray(bo[:, None]),
        "g1vv": np.ascontiguousarray(g1w[:, :C].T / HW_N),
        "g1vh": np.ascontiguousarray(g1w[:, C : 2 * C].T / HW_N),
        "g1st": np.ascontiguousarray(g1w[:, 2 * C : 2 * C + 3].T * np.array([1.0 / HW_N, 1.0, 1.0], f32)[:, None]),
        "g1b": np.ascontiguousarray(g1b[None, :]),
        "g2w": np.ascontiguousarray(g2w.T), "g2b": np.ascontiguousarray(g2b[None, :]),
        "g3w": np.ascontiguousarray(g3w.T), "g3b": np.ascontiguousarray(g3b[None, :]),
    }
    in_maps = []
    for b in range(B):
        m = dict(shared)
        m["x_vv"] = np.ascontiguousarray(vv[b].reshape(C, HW_N))
        m["x_vh"] = np.ascontiguousarray(vh[b].reshape(C, HW_N))
        m["coh"] = np.ascontiguousarray(coh[b].reshape(1, HW_N))
        in_maps.append(m)
    return in_maps


def kernel(**inputs) -> np.ndarray:
    nc = build_program()
    in_maps = make_in_maps(inputs)
    from concourse.bass_utils import run_bass_kernel_spmd

    res = run_bass_kernel_spmd(nc, in_maps, core_ids=list(range(NCORES)))
    out = np.stack([res.results[i]["y"].reshape(C, 32, 32) for i in range(B)])
    return np.ascontiguousarray(out.astype(np.float32))


# revision 14
# speedup vs baseline: 3.8635x; 1.0251x over previous
"""CoherenceGuidedAttention Trainium2 Bass kernel.

Sharding: data-parallel over batch B=8 -> one batch per NeuronCore (8 cores).
All parameters replicated; no collectives.

Per-core algorithm (flash-style, channel-major "layout B"):
  X_vv, X_vh       : [C=128, N=1024] SBUF (natural layout of [C,H,W] input)
  Q^T = temp*(Wq X_vv + bq), K^T = Wk X_vh + bk   (pair-padded head layout)
  V'  = (X_vh^T Wv + bv | ones)  token-major with per-head 17-col groups
  S^T(h) = K_h Q_h^T   computed per (head, m-tile) into PSUM [128,1024]
           + rank-1 accumulate  c*w w^T  (c=(1-g)/g) via K=1 matmuls
  expS = exp(g * S^T_blend)  on ACT directly from PSUM (scale = per-partition g)
  PV   = V'_h^T expS  accumulated over m-chunks -> [17, 512] (row 16 = denom)
  Aout^T[h] = PV[0:16] * (1/denom)  (fast reciprocal + partition broadcast)
  Y^T = Wo Aout^T + bo + X_vv  -> output [C, N]

Head layout for PE row-tiling: Q^T/K^T stored as 4 pair-tiles [128, 1024];
pair t holds head 2t at partitions 0:16 and head 2t+1 at partitions 64:80,
so the two heads' QK^T matmuls run concurrently in PE row-strips 0 and 64.
"""

import sys

if "/opt/trn_rl_repo" not in sys.path:
    sys.path.insert(0, "/opt/trn_rl_repo")

import numpy as np

B = 8
C = 128
HW_N = 1024  # H*W tokens
NH = 8       # heads
HD = 16      # head dim
NCORES = 8
NPAIR = 4    # head pairs
MT = 8       # m tiles of 128 tokens
NCHUNK = 512  # fp32 moving-operand max

_CACHE: dict = {}


def build_program():
    """Build (once) the SPMD Bass program for one core."""
    if "nc" in _CACHE:
        return _CACHE["nc"]

    from concourse import bacc
    import concourse.mybir as mybir
    import concourse.tile as tile

    f32 = mybir.dt.float32
    f32r = mybir.dt.float32r
    bf16 = mybir.dt.bfloat16
    AF = mybir.ActivationFunctionType
    OP = mybir.AluOpType

    nc = bacc.Bacc("TRN2", target_bir_lowering=False, debug=False)

    # ---- DRAM I/O ----------------------------------------------------------
    x_vv_d = nc.dram_tensor("x_vv", [C, HW_N], f32, kind="ExternalInput")
    x_vh_d = nc.dram_tensor("x_vh", [C, HW_N], f32, kind="ExternalInput")
    coh_d = nc.dram_tensor("coh", [1, HW_N], f32, kind="ExternalInput")
    wq_d = nc.dram_tensor("wq", [C, 2 * C], f32, kind="ExternalInput")
    wk_d = nc.dram_tensor("wk", [C, 2 * C], f32, kind="ExternalInput")
    bq_d = nc.dram_tensor("bq", [C, 2], f32, kind="ExternalInput")
    bk_d = nc.dram_tensor("bk", [C, 2], f32, kind="ExternalInput")
    wv_d = nc.dram_tensor("wv", [C, C], f32, kind="ExternalInput")
    bv_d = nc.dram_tensor("bv", [C, C], f32, kind="ExternalInput")
    woa_d = nc.dram_tensor("woa", [C, C], f32, kind="ExternalInput")
    wob_d = nc.dram_tensor("wob", [C, C], f32, kind="ExternalInput")
    bo_d = nc.dram_tensor("bo", [C, 1], f32, kind="ExternalInput")
    g1vv_d = nc.dram_tensor("g1vv", [C, 64], f32, kind="ExternalInput")
    g1vh_d = nc.dram_tensor("g1vh", [C, 64], f32, kind="ExternalInput")
    g1st_d = nc.dram_tensor("g1st", [3, 64], f32, kind="ExternalInput")
    g1b_d = nc.dram_tensor("g1b", [1, 64], f32, kind="ExternalInput")
    g2w_d = nc.dram_tensor("g2w", [64, 32], f32, kind="ExternalInput")
    g2b_d = nc.dram_tensor("g2b", [1, 32], f32, kind="ExternalInput")
    g3w_d = nc.dram_tensor("g3w", [32, 1], f32, kind="ExternalInput")
    g3b_d = nc.dram_tensor("g3b", [1, 1], f32, kind="ExternalInput")
    y_d = nc.dram_tensor("y", [C, HW_N], f32, kind="ExternalOutput")

    with tile.TileContext(nc) as tc:
        with (
            tc.tile_pool(name="persist", bufs=1) as persist,
            tc.tile_pool(name="qkbuf", bufs=1) as qkbuf,
            tc.tile_pool(name="expp", bufs=6) as expp,
            tc.tile_pool(name="small", bufs=8) as small,
            tc.tile_pool(name="rdp", bufs=4) as rdp,
            tc.tile_pool(name="rdbp", bufs=4) as rdbp,
            tc.tile_pool(name="ps_s", bufs=3, space="PSUM") as ps_s,
            tc.tile_pool(name="ps_pv", bufs=2, space="PSUM") as ps_pv,
        ):
            # ---- load inputs ------------------------------------------------
            xvv = persist.tile([C, HW_N], f32)
            xvh = persist.tile([C, HW_N], f32)
            coh = persist.tile([1, HW_N], f32)
            wq = persist.tile([C, 2, C], f32)
            wk = persist.tile([C, 2, C], f32)
            bq = persist.tile([C, 2], f32)
            bk = persist.tile([C, 2], f32)
            wv = persist.tile([C, C], f32)
            bvb = persist.tile([C, C], f32)
            woa = persist.tile([C, C], f32)
            wob = persist.tile([C, C], f32)
            bo = persist.tile([C, 1], f32)
            g1vv = persist.tile([C, 64], f32)
            g1vh = persist.tile([C, 64], f32)
            g1st = persist.tile([3, 64], f32)
            g1b = persist.tile([1, 64], f32)
            g2w = persist.tile([64, 32], f32)
            g2b = persist.tile([1, 32], f32)
            g3w = persist.tile([32, 1], f32)
            g3b = persist.tile([1, 1], f32)

            nc.sync.dma_start(out=xvv, in_=x_vv_d[:, :])
            nc.sync.dma_start(out=xvh, in_=x_vh_d[:, :])
            nc.sync.dma_start(out=coh, in_=coh_d[:, :])
            nc.sync.dma_start(out=wq, in_=wq_d.ap().rearrange("p (t m) -> p t m", t=2))
            nc.sync.dma_start(out=wk, in_=wk_d.ap().rearrange("p (t m) -> p t m", t=2))
            nc.sync.dma_start(out=bq, in_=bq_d[:, :])
            nc.sync.dma_start(out=bk, in_=bk_d[:, :])
            nc.sync.dma_start(out=wv, in_=wv_d[:, :])
            nc.sync.dma_start(out=bvb, in_=bv_d[:, :])
            nc.sync.dma_start(out=woa, in_=woa_d[:, :])
            nc.sync.dma_start(out=wob, in_=wob_d[:, :])
            nc.sync.dma_start(out=bo, in_=bo_d[:, :])
            nc.sync.dma_start(out=g1vv, in_=g1vv_d[:, :])
            nc.sync.dma_start(out=g1vh, in_=g1vh_d[:, :])
            nc.sync.dma_start(out=g1st, in_=g1st_d[:, :])
            nc.sync.dma_start(out=g1b, in_=g1b_d[:, :])
            nc.sync.dma_start(out=g2w, in_=g2w_d[:, :])
            nc.sync.dma_start(out=g2b, in_=g2b_d[:, :])
            nc.sync.dma_start(out=g3w, in_=g3w_d[:, :])
            nc.sync.dma_start(out=g3b, in_=g3b_d[:, :])

            ones = persist.tile([1, C], f32)
            nc.vector.memset(ones, 1.0)
            one1 = ones[:, 0:1]
            ones_col = persist.tile([C, 1], f32)
            nc.vector.memset(ones_col, 1.0)
            zero_col = persist.tile([C, 1], f32)
            nc.vector.memset(zero_col, 0.0)

            # rounded f32r views for the f32r projection matmuls
            xvv_r = persist.tile([C, HW_N], f32r)
            nc.gpsimd.tensor_copy(out=xvv_r, in_=xvv)
            xvh_r = persist.tile([C, HW_N], f32r)
            nc.gpsimd.tensor_copy(out=xvh_r, in_=xvh)
            wq_r = persist.tile([C, 2, C], f32r)
            nc.vector.tensor_copy(out=wq_r, in_=wq)
            wk_r = persist.tile([C, 2, C], f32r)
            nc.vector.tensor_copy(out=wk_r, in_=wk)
            wv_r = persist.tile([C, C], f32r)
            nc.vector.tensor_copy(out=wv_r, in_=wv)

            # ---- coherence weights w + stats -------------------------------
            cmx = small.tile([1, 1], f32)
            nc.vector.tensor_reduce(out=cmx, in_=coh, axis=mybir.AxisListType.X, op=OP.max)
            cmn = small.tile([1, 1], f32)
            nc.vector.tensor_reduce(out=cmn, in_=coh, axis=mybir.AxisListType.X, op=OP.min)
            rng = small.tile([1, 1], f32)
            # (cmax + 1e-8) - cmin
            nc.vector.scalar_tensor_tensor(
                out=rng, in0=cmx, scalar=1e-8, in1=cmn, op0=OP.add, op1=OP.subtract
            )
            rcp = small.tile([1, 1], f32)
            nc.vector.reciprocal(out=rcp, in_=rng)
            w_sb = persist.tile([1, HW_N], f32)
            nc.vector.tensor_scalar(
                out=w_sb, in0=coh, scalar1=cmn, scalar2=rcp, op0=OP.subtract, op1=OP.mult
            )

            # stats: s1 = sum(w), s2 = sum(w^2), wmx = max(w)
            s1 = small.tile([1, 1], f32)
            nc.vector.tensor_reduce(out=s1, in_=w_sb, axis=mybir.AxisListType.X, op=OP.add)
            w2 = small.tile([1, HW_N], f32, tag="wrow")
            nc.vector.tensor_mul(out=w2, in0=w_sb, in1=w_sb)
            s2 = small.tile([1, 1], f32)
            nc.vector.tensor_reduce(out=s2, in_=w2, axis=mybir.AxisListType.X, op=OP.add)
            wmx = small.tile([1, 1], f32)
            nc.vector.tensor_reduce(out=wmx, in_=w_sb, axis=mybir.AxisListType.X, op=OP.max)
            # std = sqrt(s2/N - (s1/N)^2) via exp(0.5*ln(var))
            m1 = small.tile([1, 1], f32)
            nc.vector.tensor_scalar_mul(out=m1, in0=s1, scalar1=1.0 / HW_N)
            msq = small.tile([1, 1], f32)
            nc.vector.tensor_mul(out=msq, in0=m1, in1=m1)
            var = small.tile([1, 1], f32)
            nc.vector.scalar_tensor_tensor(
                out=var, in0=s2, scalar=1.0 / HW_N, in1=msq, op0=OP.mult, op1=OP.subtract
            )
            lnv = small.tile([1, 1], f32)
            nc.scalar.activation(out=lnv, in_=var, func=AF.Ln)
            std = small.tile([1, 1], f32)
            nc.scalar.activation(out=std, in_=lnv, func=AF.Exp, scale=0.5)
            # stats row [1,3] = [s1, std, max]  (g1st row 0 is pre-scaled by 1/N)
            strow = small.tile([1, 3], f32)
            nc.vector.tensor_copy(out=strow[:, 0:1], in_=s1)
            nc.vector.tensor_copy(out=strow[:, 1:2], in_=std)
            nc.vector.tensor_copy(out=strow[:, 2:3], in_=wmx)
            ps_st = ps_pv.tile([C, NCHUNK], f32, tag="pv")
            nc.tensor.matmul(out=ps_st[0:3, 0:1], lhsT=strow, rhs=one1,
                             start=True, stop=True, tile_position=(0, 0))
            stcol = small.tile([3, 1], f32)
            nc.vector.tensor_copy(out=stcol, in_=ps_st[0:3, 0:1])

            # ---- gate MLP ---------------------------------------------------
            vvs = small.tile([C, 1], f32, tag="col")
            nc.vector.tensor_reduce(out=vvs, in_=xvv, axis=mybir.AxisListType.X, op=OP.add)
            vhs = small.tile([C, 1], f32, tag="col")
            nc.vector.tensor_reduce(out=vhs, in_=xvh, axis=mybir.AxisListType.X, op=OP.add)

            ps_g = ps_pv.tile([C, NCHUNK], f32, tag="pv")
            h1p = ps_g[0:64, 0:1]
            nc.tensor.matmul(out=h1p, lhsT=g1vv, rhs=vvs, start=True, stop=False,
                             tile_position=(0, 0))
            nc.tensor.matmul(out=h1p, lhsT=g1vh, rhs=vhs, start=False, stop=False,
                             tile_position=(0, 0))
            nc.tensor.matmul(out=h1p, lhsT=g1st, rhs=stcol, start=False, stop=False,
                             tile_position=(0, 0))
            nc.tensor.matmul(out=h1p, lhsT=g1b, rhs=one1, start=False, stop=True,
                             tile_position=(0, 0))
            h1 = small.tile([64, 1], f32, tag="col")
            nc.scalar.activation(out=h1, in_=h1p, func=AF.Relu)

            ps_g2 = ps_pv.tile([C, NCHUNK], f32, tag="pv")
            h2p = ps_g2[0:32, 0:1]
            nc.tensor.matmul(out=h2p, lhsT=g2w, rhs=h1, start=True, stop=False,
                             tile_position=(0, 0))
            nc.tensor.matmul(out=h2p, lhsT=g2b, rhs=one1, start=False, stop=True,
                             tile_position=(0, 0))
            h2 = small.tile([32, 1], f32, tag="col")
            nc.scalar.activation(out=h2, in_=h2p, func=AF.Relu)

            ps_g3 = ps_pv.tile([C, NCHUNK], f32, tag="pv")
            zp = ps_g3[0:1, 0:1]
            nc.tensor.matmul(out=zp, lhsT=g3w, rhs=h2, start=True, stop=False,
                             tile_position=(0, 0))
            nc.tensor.matmul(out=zp, lhsT=g3b, rhs=one1, start=False, stop=True,
                             tile_position=(0, 0))
            # g = sigmoid(z) = 1/(1+exp(-z))
            ez = small.tile([1, 1], f32)
            nc.scalar.activation(out=ez, in_=zp, func=AF.Exp, scale=-1.0)
            gden = small.tile([1, 1], f32)
            nc.vector.tensor_scalar_add(out=gden, in0=ez, scalar1=1.0)
            gsc = small.tile([1, 1], f32)
            nc.vector.reciprocal(out=gsc, in_=gden)
            # broadcast g to all partitions
            ps_gb = ps_pv.tile([C, NCHUNK], f32, tag="pv")
            nc.tensor.matmul(out=ps_gb[:, 0:1], lhsT=ones, rhs=gsc,
                             start=True, stop=True, tile_position=(0, 0))
            g_col = persist.tile([C, 1], f32)
            nc.vector.tensor_copy(out=g_col, in_=ps_gb[:, 0:1])
            # c = (1-g)/g = 1/g - 1  (scalar, partition 0); wc_row = c*w
            rg1 = small.tile([1, 1], f32)
            nc.vector.reciprocal(out=rg1, in_=gsc)
            c1 = small.tile([1, 1], f32)
            nc.vector.tensor_scalar_add(out=c1, in0=rg1, scalar1=-1.0)
            wc_row = persist.tile([1, HW_N], bf16)
            nc.vector.tensor_scalar_mul(out=wc_row, in0=w_sb, scalar1=c1)
            w_r = persist.tile([1, HW_N], bf16)
            nc.vector.tensor_copy(out=w_r, in_=w_sb)

            # ---- V' projection (token-major, 17-col head groups + ones) ----
            # vp[p, mc, h, 0] = 1;  vp[p, mc, h, 1:17] = V_seq[mc*128+p, 16h:16h+16]
            # (ones first so the PV denominator row lands on the 32-aligned
            #  strip base - engine APs must start at partition 0/32/64/96)
            vp = persist.tile([C, MT, NH, HD + 1], bf16)
            # memset can't write f32r; broadcast-copy rounds f32 -> f32r
            nc.vector.tensor_copy(out=vp[:, :, :, 0:1],
                                  in_=ones_col.to_broadcast([C, MT, NH, 1]))
            for gp in range(2):
                ps_v = ps_pv.tile([C, NCHUNK], f32, tag="pv")
                for i in range(4):
                    mc = 4 * gp + i
                    nc.tensor.matmul(
                        out=ps_v[:, i * C : (i + 1) * C],
                        lhsT=xvh_r[:, mc * C : (mc + 1) * C],
                        rhs=wv_r,
                        start=True, stop=True, tile_position=(0, 0),
                    )
                for i in range(4):
                    mc = 4 * gp + i
                    nc.vector.tensor_add(
                        out=vp[:, mc, :, 1 : HD + 1],
                        in0=ps_v[:, i * C : (i + 1) * C].rearrange(
                            "p (h d) -> p h d", h=NH
                        ),
                        in1=bvb.rearrange("p (h d) -> p h d", h=NH),
                    )

            # ---- Q^T / K^T projections (pair-padded head layout) -----------
            qt = qkbuf.tile([C, 2, HW_N], bf16)
            kt = qkbuf.tile([C, 2, HW_N], bf16)
            for g in range(2):
                ps_q = ps_s.tile([C, HW_N], f32, tag="s")
                for ncb in range(2):
                    sl = slice(ncb * NCHUNK, (ncb + 1) * NCHUNK)
                    nc.tensor.matmul(out=ps_q[:, sl], lhsT=wq_r[:, g, :],
                                     rhs=xvv_r[:, sl],
                                     start=True, stop=True, tile_position=(0, 0))
                nc.scalar.activation(out=qt[:, g, :], in_=ps_q, func=AF.Identity,
                                     bias=bq[:, g : g + 1], scale=1.0)
            for g in range(2):
                ps_k = ps_s.tile([C, HW_N], f32, tag="s")
                for ncb in range(2):
                    sl = slice(ncb * NCHUNK, (ncb + 1) * NCHUNK)
                    nc.tensor.matmul(out=ps_k[:, sl], lhsT=wk_r[:, g, :],
                                     rhs=xvh_r[:, sl],
                                     start=True, stop=True, tile_position=(0, 0))
                nc.scalar.activation(out=kt[:, g, :], in_=ps_k, func=AF.Identity,
                                     bias=bk[:, g : g + 1], scale=1.0)

            # The blended scores are computed as one K=17 contraction:
            # row 16 of each strip of kt holds w, of qt holds c*w, so
            # S^T + c*w w^T comes out of a single matmul. Engine writes
            # can't target partition 16/80 (alignment), but DMA can.
            for g in range(2):
                for j in range(4):
                    s_ = 32 * j + HD
                    nc.sync.dma_start(out=kt[s_ : s_ + 1, g, :], in_=w_r)
                    nc.sync.dma_start(out=qt[s_ : s_ + 1, g, :], in_=wc_row)

            # ---- attention: S^T -> exp -> PV, flash-style ------------------
            aoutA = persist.tile([C, HW_N], bf16)
            aoutB = persist.tile([C, HW_N], bf16)
            nc.vector.tensor_copy(out=aoutA, in_=zero_col.to_broadcast([C, HW_N]))
            nc.vector.tensor_copy(out=aoutB, in_=zero_col.to_broadcast([C, HW_N]))
            for g in range(2):
                # one PV psum tile per n-chunk holds all 4 heads of the quad
                # at col strips 0/32/64/96 (4-way col-tiled PV matmuls).
                pv_tiles = {
                    ncb: ps_pv.tile(
                        [C, NCHUNK], f32, tag="pv", name=f"pv_{g}_{ncb}"
                    )
                    for ncb in range(2)
                }
                for mt_i in range(MT):
                    msl = slice(mt_i * C, (mt_i + 1) * C)
                    for ncb in range(2):
                        sl = slice(ncb * NCHUNK, (ncb + 1) * NCHUNK)
                        # two psum tiles per wave; each holds two heads'
                        # n-chunk so one exp unblocks two PV matmuls
                        pshalf = []
                        for half in range(2):
                            pshalf.append(ps_s.tile(
                                [C, HW_N], f32, tag="s",
                                name=f"s_{g}_{mt_i}_{ncb}_{half}"))
                        for j in range(4):
                            nc.tensor.matmul(
                                out=pshalf[j // 2][:, (j % 2) * NCHUNK
                                                   : (j % 2 + 1) * NCHUNK],
                                lhsT=kt[32 * j : 32 * j + HD + 1, g, msl],
                                rhs=qt[32 * j : 32 * j + HD + 1, g, sl],
                                start=True, stop=True, tile_position=(32 * j, 0),
                            )
                        es_tiles = []
                        for half in range(2):
                            es = expp.tile([C, HW_N], bf16, tag="es",
                                           name=f"es_{half}")
                            nc.scalar.activation(out=es, in_=pshalf[half],
                                                 func=AF.Exp, scale=g_col)
                            es_tiles.append(es)
                        for j in range(4):
                            nc.tensor.matmul(
                                out=pv_tiles[ncb][32 * j : 32 * j + HD + 1, :],
                                lhsT=vp[:, mt_i, 4 * g + j, :],
                                rhs=es_tiles[j // 2][:, (j % 2) * NCHUNK
                                                     : (j % 2 + 1) * NCHUNK],
                                start=(mt_i == 0), stop=(mt_i == MT - 1),
                                tile_position=(0, 32 * j),
                                skip_group_check=True,
                            )
                # normalize the quad's four heads
                for j in range(4):
                    h = 4 * g + j
                    for ncb in range(2):
                        pvt = pv_tiles[ncb]
                        rd = rdp.tile([1, NCHUNK], f32, tag="rd")
                        nc.vector.reciprocal_approx_fast(
                            out=rd, in_=pvt[32 * j : 32 * j + 1, :]
                        )
                        rdb = rdbp.tile([HD + 1, NCHUNK], f32, tag="rdb")
                        nc.gpsimd.partition_broadcast(rdb, rd)
                        # denom*recip(denom) junk row hits a zero Wo row.
                        dst = aoutA if g == 0 else aoutB
                        nc.vector.tensor_mul(
                            out=dst[32 * j : 32 * j + HD + 1,
                                    ncb * NCHUNK : (ncb + 1) * NCHUNK],
                            in0=pvt[32 * j : 32 * j + HD + 1, :],
                            in1=rdb,
                        )

            # rounded copies of the padded output-projection weights
            woa_r = persist.tile([C, C], bf16)
            nc.vector.tensor_copy(out=woa_r, in_=woa)
            wob_r = persist.tile([C, C], bf16)
            nc.vector.tensor_copy(out=wob_r, in_=wob)

            # ---- output projection + bias + residual -----------------------
            ps_y = ps_s.tile([C, HW_N], f32, tag="s")
            for ncb in range(2):
                sl = slice(ncb * NCHUNK, (ncb + 1) * NCHUNK)
                nc.tensor.matmul(out=ps_y[:, sl], lhsT=woa_r, rhs=aoutA[:, sl],
                                 start=True, stop=False, tile_position=(0, 0))
                nc.tensor.matmul(out=ps_y[:, sl], lhsT=wob_r, rhs=aoutB[:, sl],
                                 start=False, stop=True, tile_position=(0, 0))
            y_sb = persist.tile([C, HW_N], f32)
            # (ps_y + bo) + x_vv
            nc.vector.scalar_tensor_tensor(
                out=y_sb, in0=ps_y, scalar=bo, in1=xvv, op0=OP.add, op1=OP.add
            )
            nc.sync.dma_start(out=y_d[:, :], in_=y_sb)

    nc.compile()
    _CACHE["nc"] = nc
    return nc


def make_in_maps(inputs: dict) -> list[dict]:
    """Host-side prep: shard over batch, pre-transpose/pad the small weights."""
    f32 = np.float32
    vv = np.ascontiguousarray(inputs["vv_features"], dtype=f32)
    vh = np.ascontiguousarray(inputs["vh_features"], dtype=f32)
    coh = np.ascontiguousarray(inputs["coherence_matrix"], dtype=f32)
    Wq = np.asarray(inputs["Wq"], f32)
    bq = np.asarray(inputs["bq"], f32)
    Wk = np.asarray(inputs["Wk"], f32)
    bk = np.asarray(inputs["bk"], f32)
    Wv = np.asarray(inputs["Wv"], f32)
    bv = np.asarray(inputs["bv"], f32)
    Wo = np.asarray(inputs["Wo"], f32)
    bo = np.asarray(inputs["bo"], f32)
    temp = float(np.asarray(inputs["temperature"], f32).reshape(-1)[0])
    g1w = np.asarray(inputs["g1w"], f32)
    g1b = np.asarray(inputs["g1b"], f32)
    g2w = np.asarray(inputs["g2w"], f32)
    g2b = np.asarray(inputs["g2b"], f32)
    g3w = np.asarray(inputs["g3w"], f32)
    g3b = np.asarray(inputs["g3b"], f32)

    def _wo_pad(Wo_, grp):
        # lhsT [c_in_padded=128, c_out=128]: strip j row d holds Wo column for
        # channel 16*(4*grp+j)+d; pad rows (d>=16) are zero.
        wp = np.zeros((C, C), f32)
        for j in range(4):
            ch0 = HD * (4 * grp + j)
            wp[32 * j + 1 : 32 * j + 1 + HD, :] = Wo_[:, ch0 : ch0 + HD].T
        return np.ascontiguousarray(wp)

    def pad_quad(Wt, bt):
        # quad g holds head 4g+j at partitions 32j..32j+16 (w row at 32j+16)
        wpad = np.zeros((2, C, C), f32)   # [g, c_in, m]
        bpad = np.zeros((C, 2), f32)
        for g in range(2):
            for j in range(4):
                h = 4 * g + j
                wpad[g, :, 32 * j : 32 * j + HD] = Wt[h * HD : (h + 1) * HD, :].T
                bpad[32 * j : 32 * j + HD, g] = bt[h * HD : (h + 1) * HD]
        wflat = np.ascontiguousarray(wpad.transpose(1, 0, 2).reshape(C, 2 * C))
        return wflat, np.ascontiguousarray(bpad)

    wq_h, bq_h = pad_quad(Wq * temp, bq * temp)
    wk_h, bk_h = pad_quad(Wk, bk)

    shared = {
        "wq": wq_h, "bq": bq_h, "wk": wk_h, "bk": bk_h,
        "wv": np.ascontiguousarray(Wv.T), "bv": np.ascontiguousarray(np.tile(bv, (C, 1))),
        "woa": _wo_pad(Wo, 0), "wob": _wo_pad(Wo, 1),
        "bo": np.ascontiguousarray(bo[:, None]),
        "g1vv": np.ascontiguousarray(g1w[:, :C].T / HW_N),
        "g1vh": np.ascontiguousarray(g1w[:, C : 2 * C].T / HW_N),
        "g1st": np.ascontiguousarray(g1w[:, 2 * C : 2 * C + 3].T * np.array([1.0 / HW_N, 1.0, 1.0], f32)[:, None]),
        "g1b": np.ascontiguousarray(g1b[None, :]),
        "g2w": np.ascontiguousarray(g2w.T), "g2b": np.ascontiguousarray(g2b[None, :]),
        "g3w": np.ascontiguousarray(g3w.T), "g3b": np.ascontiguousarray(g3b[None, :]),
    }
    in_maps = []
    for b in range(B):
        m = dict(shared)
        m["x_vv"] = np.ascontiguousarray(vv[b].reshape(C, HW_N))
        m["x_vh"] = np.ascontiguousarray(vh[b].reshape(C, HW_N))
        m["coh"] = np.ascontiguousarray(coh[b].reshape(1, HW_N))
        in_maps.append(m)
    return in_maps


def kernel(**inputs) -> np.ndarray:
    nc = build_program()
    in_maps = make_in_maps(inputs)
    from concourse.bass_utils import run_bass_kernel_spmd

    res = run_bass_kernel_spmd(nc, in_maps, core_ids=list(range(NCORES)))
    out = np.stack([res.results[i]["y"].reshape(C, 32, 32) for i in range(B)])
    return np.ascontiguousarray(out.astype(np.float32))
